# revision 1
# baseline (speedup 1.0000x reference)
"""CrystalGraphALIGNN Trainium2 kernel (8 NeuronCores, SPMD).

Strategy: dst-shard edges across cores (atom v owned by core v // (N/8); edge
(i,j) owned by the core of its dst). Per core, edges are sorted by dst and
grouped into 128-atom blocks so that:
  - the dst-side expansion A_dst[dst(e)] is a block-local one-hot matmul
    (S^T streamed from DRAM),
  - the scatter-mean aggregation is a one-hot matmul into PSUM (S streamed),
  - only the src side needs a true random gather: per-edge rows of
    A_src = node @ W_src, fetched with dma_gather (transposed, bf16) from a
    DRAM table that is refreshed once per layer via AllGather.
Node states and the node MLP stay fully shard-local; crystal pooling is a
one-hot matmul + a single AllReduce, readout replicated on every core.
"""

import numpy as np
import ml_dtypes

import concourse.bass as bass
import concourse.bacc as bacc
import concourse.mybir as mybir
import concourse.tile as tile
from concourse import library_config

F32 = mybir.dt.float32
BF16 = mybir.dt.bfloat16
I16 = mybir.dt.int16
AFT = mybir.ActivationFunctionType
BF = ml_dtypes.bfloat16

NCORES = 8
ED, ND, HID, RD = 64, 128, 128, 128
EDGE_THRESH = 1e-6
GC = 125  # crystals per pooling group

FULL_CFG = dict(N=50000, M=12, AFD=92, EFD=41, NCRYS=1000, L=4)


def _cdiv(a, b):
    return (a + b - 1) // b


def _wrap_idx(flat):
    """int16 flat idx -> [128, len/16] wrapped+replicated layout."""
    n = len(flat)
    assert n % 16 == 0
    w = flat.reshape(n // 16, 16).T.astype(np.int16)
    return np.tile(w, (8, 1))


def _prep(inputs, cfg):
    N, M, AFD, EFD, NCRYS, L = (cfg[k] for k in ("N", "M", "AFD", "EFD", "NCRYS", "L"))
    ASH = N // NCORES
    NBLK = _cdiv(ASH, 128)
    LOS = min(25000, N)  # src index split for int16 gather indices
    NG = _cdiv(NCRYS, GC)

    af = np.asarray(inputs["atom_fea"], np.float32)
    nf = np.asarray(inputs["nbr_fea"], np.float32)
    nidx = np.asarray(inputs["nbr_fea_idx"]).astype(np.int64)
    cb = np.asarray(inputs["crystal_batch"]).astype(np.int64)

    E = N * M
    dst = np.clip(nidx.reshape(-1), 0, N - 1)
    src = np.repeat(np.arange(N, dtype=np.int64), M)
    ea = nf.reshape(E, EFD)
    mask = (np.abs(ea).sum(1) > EDGE_THRESH).astype(np.float32)

    cnt = np.bincount(dst, weights=mask, minlength=N)
    invcnt = (1.0 / np.maximum(cnt, 1.0)).astype(np.float32)
    ccnt = np.bincount(cb, minlength=NCRYS).astype(np.float32)
    invccnt = (1.0 / np.maximum(ccnt, 1.0)).astype(np.float32)

    core_of = dst // ASH
    dloc = dst - core_of * ASH
    blk_of = dloc // 128

    # per-core, per-block, lo/hi edge id lists
    lists = [[[None, None] for _ in range(NBLK)] for _ in range(NCORES)]
    order = np.lexsort((dst, blk_of + core_of * NBLK))  # group by (core, blk)
    for k in range(NCORES):
        esel = order[(core_of[order] == k)]
        for b in range(NBLK):
            eb = esel[blk_of[esel] == b]
            lists[k][b][0] = eb[src[eb] < LOS]
            lists[k][b][1] = eb[src[eb] >= LOS]

    T_lo = np.zeros(NBLK, np.int64)
    T_hi = np.zeros(NBLK, np.int64)
    for b in range(NBLK):
        T_lo[b] = max(_cdiv(max(len(lists[k][b][0]) for k in range(NCORES)), 128), 1)
        T_hi[b] = _cdiv(max(len(lists[k][b][1]) for k in range(NCORES)), 128)
        if (T_lo[b] + T_hi[b]) % 2:
            if N > LOS:
                T_hi[b] += 1
            else:
                T_lo[b] += 1

    # geometry: edge-col space (block-major), state-col space (per half),
    # chunk list entries: (b, half, state_col, edge_col, blk_edge_col, n)
    BHALF = NBLK // 2
    ecol = np.zeros(NBLK + 1, np.int64)
    for b in range(NBLK):
        ecol[b + 1] = ecol[b] + (T_lo[b] + T_hi[b]) * 128
    EP = int(ecol[NBLK])
    scol = np.zeros(NBLK, np.int64)
    acc = [0, 0]
    blocks = []
    for b in range(NBLK):
        half = 0 if b < BHALF else 1
        scol[b] = acc[half]
        nb_e = (T_lo[b] + T_hi[b]) * 128
        acc[half] += nb_e
        tiles = (T_lo[b] + T_hi[b])
        chunks = []
        off = 0
        while tiles > 0:
            t = 4 if tiles >= 4 else tiles
            chunks.append((int(scol[b] + off), int(ecol[b] + off), off, t * 128))
            off += t * 128
            tiles -= t
        blocks.append(dict(b=b, half=half, nblk_e=nb_e, chunks=chunks,
                           n_lo=int(T_lo[b] * 128), n_hi=int(T_hi[b] * 128)))
    EPC = max(acc)
    IWL = sum(int(t) * 8 for t in T_lo)
    IWH = sum(int(t) * 8 for t in T_hi)

    meta = dict(cfg=cfg, ASH=ASH, NBLK=NBLK, LOS=LOS, NG=NG, EP=EP, EPC=EPC,
                BHALF=BHALF, blocks=blocks, IWL=IWL, IWH=IWH,
                out_b=float(np.asarray(inputs["out_b"]).reshape(-1)[0]))

    # shared weights
    eW1 = np.asarray(inputs["eW1"], np.float32)
    eW2 = np.asarray(inputs["eW2"], np.float32)
    nW1 = np.asarray(inputs["nW1"], np.float32)
    nW2 = np.asarray(inputs["nW2"], np.float32)

    def bfc(x):
        return np.ascontiguousarray(x, np.float32).astype(BF)

    atomW93 = np.zeros((AFD + 1, ND), np.float32)
    atomW93[:AFD] = np.asarray(inputs["atom_W"], np.float32)
    atomW93[AFD] = np.asarray(inputs["atom_b"], np.float32)
    edgeW42 = np.zeros((EFD + 1, ED), np.float32)
    edgeW42[:EFD] = np.asarray(inputs["edge_W"], np.float32)
    edgeW42[EFD] = np.asarray(inputs["edge_b"], np.float32)

    we_dup = np.zeros((128, L * HID), np.float32)
    nw1b_dup = np.zeros((128, L * ND), np.float32)
    for l in range(L):
        we_dup[0:64, l * HID:(l + 1) * HID] = eW1[l, 0:ED]
        we_dup[64:128, l * HID:(l + 1) * HID] = eW1[l, 0:ED]
        nw1b_dup[0:64, l * ND:(l + 1) * ND] = nW1[l, ND:ND + ED]
        nw1b_dup[64:128, l * ND:(l + 1) * ND] = nW1[l, ND:ND + ED]
    ws_all = np.concatenate([eW1[l, ED:ED + ND] for l in range(L)], 1)      # [128, L*128]
    wd_all = np.concatenate([eW1[l, ED + ND:] for l in range(L)], 1)        # [128, L*128]
    ew2_all = np.concatenate([eW2[l] for l in range(L)], 1)                 # [128, L*64]
    nw1a_all = np.concatenate([nW1[l, 0:ND] for l in range(L)], 1)          # [128, L*128]
    nw2_all = np.concatenate([nW2[l] for l in range(L)], 1)                 # [128, L*128]

    eb1 = np.asarray(inputs["eb1"], np.float32).T.copy()                    # [128, L]
    eb2p = np.zeros((128, L), np.float32)
    eb2p[0:64] = np.asarray(inputs["eb2"], np.float32).T
    eb2p[64:128] = eb2p[0:64]
    nb1 = np.asarray(inputs["nb1"], np.float32).T.copy()
    nb2 = np.asarray(inputs["nb2"], np.float32).T.copy()

    i64d = np.zeros((128, 64), np.float32)
    i64d[0:64] = np.eye(64)
    i64d[64:128] = np.eye(64)

    eb2rep = np.repeat(eb2p[:, :, None], 512, axis=2).reshape(128, L * 512)
    shared = {
        "eb2rep": np.ascontiguousarray(eb2rep, np.float32).astype(BF),
        "atomW": bfc(atomW93), "edgeW": bfc(edgeW42),
        "we_dup": bfc(we_dup), "nw1b_dup": bfc(nw1b_dup),
        "ws_all": bfc(ws_all), "wd_all": bfc(wd_all), "ew2_all": bfc(ew2_all),
        "nw1a_all": bfc(nw1a_all), "nw2_all": bfc(nw2_all),
        "readW": bfc(np.asarray(inputs["read_W"])), "outW": bfc(np.asarray(inputs["out_W"])),
        "eb1": eb1, "eb2p": eb2p, "nb1": nb1, "nb2": nb2,
        "readb": np.asarray(inputs["read_b"], np.float32).reshape(RD, 1),
        "i64d": bfc(i64d), "i128b": bfc(np.eye(128)), "i128f": np.eye(128, dtype=np.float32),
        "invccnt": np.pad(invccnt, (0, NG * GC - NCRYS)).reshape(NG, GC).T.copy(),  # [125, NG]
    }

    in_maps = []
    for k in range(NCORES):
        a0 = k * ASH
        eattrT = np.zeros((EFD + 1, EP), np.float32)
        ss = np.zeros((128, 2 * EP), np.float32)
        idxl = np.zeros(sum(int(t) * 128 for t in T_lo), np.int64)
        idxh = np.zeros(sum(int(t) * 128 for t in T_hi), np.int64)
        ol = oh = 0
        for blk in blocks:
            b = blk["b"]
            ids_lo, ids_hi = lists[k][b]
            n_lo, n_hi = blk["n_lo"], blk["n_hi"]
            eo = int(ecol[b])
            ids = np.full(n_lo + n_hi, -1, np.int64)
            ids[:len(ids_lo)] = ids_lo
            ids[n_lo:n_lo + len(ids_hi)] = ids_hi
            real = ids >= 0
            rids = ids[real]
            eattrT[:EFD, eo:eo + n_lo + n_hi][:, real] = ea[rids].T
            eattrT[EFD, eo:eo + n_lo + n_hi][real] = 1.0
            # one-hot dst (block-local) for real edges
            arow = (dloc[rids] - 128 * b)
            epos = np.nonzero(real)[0]
            for (sco, eco, bco, n) in blk["chunks"]:
                insel = (epos >= bco) & (epos < bco + n)
                ep_c = epos[insel] - bco
                ar_c = arow[insel]
                m_c = mask[rids[insel]]
                st = ss[:, 2 * eco:2 * eco + 2 * n]
                st[ar_c, ep_c] = 1.0                      # S^T [a, e]
                j = ep_c // 128
                st[ep_c % 128, n + j * 128 + ar_c] = m_c  # S tiles [e, a]
            gl = np.zeros(n_lo, np.int64)
            gl[:len(ids_lo)] = src[ids_lo]
            gh = np.zeros(n_hi, np.int64)
            gh[:len(ids_hi)] = src[ids_hi] - LOS
            idxl[ol:ol + n_lo] = gl
            idxh[oh:oh + n_hi] = gh
            ol += n_lo
            oh += n_hi

        inv_sb = np.ones((128, NBLK), np.float32)
        for b in range(NBLK):
            na = min(128, ASH - 128 * b)
            inv_sb[0:na, b] = invcnt[a0 + 128 * b: a0 + 128 * b + na]
        afT = np.zeros((AFD + 1, ASH), np.float32)
        afT[:AFD] = af[a0:a0 + ASH].T
        afT[AFD] = 1.0
        pmat = np.zeros((128, NBLK * NG * GC), np.float32)
        for b in range(NBLK):
            na = min(128, ASH - 128 * b)
            crys = cb[a0 + 128 * b: a0 + 128 * b + na]
            pmat[np.arange(na), b * NG * GC + crys] = 1.0

        m = {
            "eattrT": eattrT.astype(BF), "ss": ss.astype(BF),
            "idxlo": _wrap_idx(idxl), "invcnt": inv_sb,
            "afT": afT.astype(BF), "pmat": pmat.astype(BF),
        }
        if IWH:
            m["idxhi"] = _wrap_idx(idxh)
        m.update(shared)
        in_maps.append(m)
    return meta, in_maps


def _build(meta, act=AFT.Silu, noop=False, no_gather=False, no_coll=False):
    cfg = meta["cfg"]
    N, M, AFD, EFD, NCRYS, L = (cfg[k] for k in ("N", "M", "AFD", "EFD", "NCRYS", "L"))
    ASH, NBLK, LOS, NG = meta["ASH"], meta["NBLK"], meta["LOS"], meta["NG"]
    EP, EPC, blocks = meta["EP"], meta["EPC"], meta["blocks"]
    IWL, IWH = meta["IWL"], meta["IWH"]

    nc = bacc.Bacc("TRN2", target_bir_lowering=False, debug=False, num_devices=NCORES,
                   num_swdge_queues=4)

    def din(name, shape, dt):
        return nc.dram_tensor(name, shape, dt, kind="ExternalInput")

    eattrT = din("eattrT", [EFD + 1, EP], BF16)
    ssd = din("ss", [128, 2 * EP], BF16)
    idxlo = din("idxlo", [128, IWL], I16)
    idxhi = din("idxhi", [128, IWH], I16) if IWH else None
    invcnt = din("invcnt", [128, NBLK], F32)
    afT = din("afT", [AFD + 1, ASH], BF16)
    pmat = din("pmat", [128, NBLK * NG * GC], BF16)
    eb2rep = nc.dram_tensor("eb2rep", [128, L * 512], BF16, kind="ExternalInput")
    wts = {}
    for nm, sh, dt in [
        ("atomW", [AFD + 1, ND], BF16), ("edgeW", [EFD + 1, ED], BF16),
        ("we_dup", [128, L * HID], BF16), ("nw1b_dup", [128, L * ND], BF16),
        ("ws_all", [ND, L * HID], BF16), ("wd_all", [ND, L * HID], BF16),
        ("ew2_all", [HID, L * ED], BF16), ("nw1a_all", [ND, L * HID], BF16),
        ("nw2_all", [HID, L * ND], BF16), ("readW", [ND, RD], BF16),
        ("outW", [RD, 1], BF16), ("eb1", [128, L], F32), ("eb2p", [128, L], F32),
        ("nb1", [128, L], F32), ("nb2", [128, L], F32), ("readb", [RD, 1], F32),
        ("i64d", [128, 64], BF16), ("i128b", [128, 128], BF16),
        ("i128f", [128, 128], F32), ("invccnt", [GC, NG], F32),
    ]:
        wts[nm] = din(nm, sh, dt)
    y = nc.dram_tensor("y", [1, NCRYS], F32, kind="ExternalOutput")

    if noop:
        with tile.TileContext(nc) as tc:
            with tc.tile_pool(name="sbz", bufs=1) as sbz:
                yz = sbz.tile([1, NCRYS], F32, tag="yz")
                nc.gpsimd.memset(yz[:], 0.0)
                nc.sync.dma_start(y[:], yz[:])
        nc.compile()
        return nc

    with tile.TileContext(nc) as tc:
        with (
            tc.tile_pool(name="persist", bufs=1) as pp,
            tc.tile_pool(name="dram", bufs=1, space="DRAM") as dp,
        ):
            nc.gpsimd.load_library(library_config.mlp)
            w = {nm: pp.tile(t.shape, t.dtype, tag=nm, name=f"w_{nm}") for nm, t in wts.items()}
            for nm, t in wts.items():
                nc.sync.dma_start(w[nm][:], t[:])
            invcnt_sb = pp.tile([128, NBLK], F32, tag="invcnt_sb")
            nc.sync.dma_start(invcnt_sb[:], invcnt[:])
            eb2rep_sb = pp.tile([128, L * 512], BF16, tag="eb2rep_sb")
            nc.sync.dma_start(eb2rep_sb[:], eb2rep[:])
            stateT = pp.tile([128, EPC], BF16, tag="stateT")
            nodeT = pp.tile([128, ASH], F32, tag="nodeT")
            nodeTb = pp.tile([128, ASH], BF16, tag="nodeTb")
            adst = pp.tile([128, NBLK * 128], BF16, tag="adst")
            aggT = pp.tile([128, _cdiv(NBLK, 2) * 128], BF16, tag="aggT")
            idxsb = pp.tile([128, IWL], I16, tag="idxsb")
            nc.sync.dma_start(idxsb[:], idxlo[:])
            if IWH:
                idxsbh = pp.tile([128, IWH], I16, tag="idxsbh")
                nc.sync.dma_start(idxsbh[:], idxhi[:])
            asrc_in = dp.tile([ASH, ND], BF16)
            asrc_fulls = [dp.tile([N, ND], BF16, addr_space="Shared", name=f"asrc_full{i}", tag=f"asrc_full{i}")
                          for i in range(L)]
            pool_in = dp.tile([NCRYS, ND], F32)
            pool_out = dp.tile([NCRYS, ND], F32, addr_space="Shared")

            def node_tables(lw, sbp, psp):
                """A_src shard -> bounce -> AllGather; A_dst blocks (layer lw)."""
                for t in range(NBLK):
                    na = min(128, ASH - 128 * t)
                    lhs = nodeTb[:, 128 * t:128 * t + na]
                    ps_s = psp.tile([128, 128], F32, tag="ps_s")
                    nc.tensor.matmul(ps_s[0:na, :], lhs, w["ws_all"][:, lw * HID:(lw + 1) * HID],
                                     start=True, stop=True)
                    asb = sbp.tile([128, 128], BF16, tag="asb")
                    nc.vector.tensor_copy(asb[0:na, :], ps_s[0:na, :])
                    nc.sync.dma_start(asrc_in[128 * t:128 * t + na, :], asb[0:na, :])
                    ps_d = psp.tile([128, 128], F32, tag="ps_d")
                    nc.tensor.matmul(ps_d[0:na, :], lhs, w["wd_all"][:, lw * HID:(lw + 1) * HID],
                                     start=True, stop=True)
                    nc.vector.tensor_copy(adst[0:na, 128 * t:128 * t + 128][:, 0:128],
                                          ps_d[0:na, :])
                if not no_coll:
                    nc.gpsimd.collective_compute(
                        "AllGather", mybir.AluOpType.bypass,
                        replica_groups=[list(range(NCORES))],
                        ins=[asrc_in[:].opt()], outs=[asrc_fulls[lw][:].opt()],
                    )

            # ---- init: projections + layer-0 tables ----
            with tc.tile_pool(name="sbi", bufs=3) as sbp, \
                 tc.tile_pool(name="psi", bufs=2, space="PSUM") as psp:
                for t in range(NBLK):
                    na = min(128, ASH - 128 * t)
                    aft = sbp.tile([AFD + 1, 128], BF16, tag="aft")
                    nc.sync.dma_start(aft[:, 0:na], afT[:, 128 * t:128 * t + na])
                    ps_n = psp.tile([128, 128], F32, tag="ps_n")
                    nc.tensor.matmul(ps_n[:, 0:na], w["atomW"][:], aft[:, 0:na],
                                     start=True, stop=True)
                    nc.vector.tensor_copy(nodeT[:, 128 * t:128 * t + na], ps_n[:, 0:na])
                    nc.vector.tensor_copy(nodeTb[:, 128 * t:128 * t + na], ps_n[:, 0:na])
                for blk in blocks:
                    hr = slice(64, 128) if blk["half"] else slice(0, 64)
                    for (sco, eco, bco, n) in blk["chunks"]:
                        eat = sbp.tile([EFD + 1, 512], BF16, tag="eat")
                        nc.sync.dma_start(eat[:, 0:n], eattrT[:, eco:eco + n])
                        ps_e = psp.tile([128, 512], F32, tag="ps_e")
                        nc.tensor.matmul(ps_e[hr, 0:n], w["edgeW"][:], eat[:, 0:n],
                                         start=True, stop=True)
                        nc.vector.tensor_copy(stateT[hr, sco:sco + n], ps_e[hr, 0:n])
                node_tables(0, sbp, psp)

            # ---- layers ----
            for l in range(L):
                with tc.tile_pool(name=f"sbe{l}", bufs=3) as sbp, \
                     tc.tile_pool(name=f"pse{l}", bufs=2, space="PSUM") as psp, \
                     tc.tile_pool(name=f"psg{l}", bufs=2, space="PSUM") as psg:
                    for blk in blocks:
                        b = blk["b"]
                        hr = slice(64, 128) if blk["half"] else slice(0, 64)
                        ba = min(128, ASH - 128 * b)
                        asrc_full = asrc_fulls[l]
                        gt = sbp.tile([128, 1, blk["nblk_e"]], BF16, tag="gt", bufs=2)
                        if blk["n_lo"] and not no_gather:
                            io = sum(bb["n_lo"] for bb in blocks[:b]) // 16
                            nc.gpsimd.dma_gather(
                                gt[:, :, 0:blk["n_lo"]], asrc_full[0:LOS, :],
                                idxsb[:, io:io + blk["n_lo"] // 16],
                                blk["n_lo"], blk["n_lo"], ND, transpose=True,
                                queue_num=(2 * b) % 4)
                        if blk["n_hi"] and not no_gather:
                            io = sum(bb["n_hi"] for bb in blocks[:b]) // 16
                            nc.gpsimd.dma_gather(
                                gt[:, :, blk["n_lo"]:], asrc_full[LOS:N, :],
                                idxsbh[:, io:io + blk["n_hi"] // 16],
                                blk["n_hi"], blk["n_hi"], ND, transpose=True,
                                queue_num=(2 * b + 1) % 4)
                        ps_agg = psg.tile([128, 64], F32, tag="agg")
                        nchunk = len(blk["chunks"])
                        e0 = blk["chunks"][0][1]
                        ssb = sbp.tile([128, 2 * blk["nblk_e"]], BF16, tag="ssb", bufs=2)
                        nc.sync.dma_start(ssb[:, 0:2 * blk["nblk_e"]],
                                          ssd[:, 2 * e0:2 * e0 + 2 * blk["nblk_e"]])
                        for ci, (sco, eco, bco, n) in enumerate(blk["chunks"]):
                            sst = ssb[:, 2 * (eco - e0):2 * (eco - e0) + 2 * n]
                            ps_h = psp.tile([128, 512], F32, tag="ph")
                            nc.tensor.matmul(ps_h[:, 0:n], adst[0:ba, 128 * b:128 * b + 128],
                                             sst[0:ba, 0:n], start=True, stop=False)  # S^T chunk
                            nc.tensor.matmul(ps_h[:, 0:n], w["we_dup"][hr, l * HID:(l + 1) * HID],
                                             stateT[hr, sco:sco + n], start=False,
                                             stop=no_gather)
                            if not no_gather:
                                nc.tensor.matmul(ps_h[:, 0:n], w["i128b"][:],
                                                 gt[:, 0, bco:bco + n], start=False, stop=True)
                            ht = sbp.tile([128, 512], BF16, tag="ht")
                            nc.scalar.activation(ht[:, 0:n], ps_h[:, 0:n], act,
                                                 bias=w["eb1"][:, l:l + 1])
                            ps_dd = psp.tile([128, 512], F32, tag="pd")
                            nc.tensor.matmul(ps_dd[hr, 0:n], w["i64d"][hr, :],
                                             stateT[hr, sco:sco + n], start=True, stop=False)
                            nc.tensor.matmul(ps_dd[hr, 0:n], w["ew2_all"][:, l * ED:(l + 1) * ED],
                                             ht[:, 0:n], start=False, stop=True)
                            nc.vector.tensor_add(stateT[hr, sco:sco + n], ps_dd[hr, 0:n],
                                                 eb2rep_sb[hr, l * 512:l * 512 + n])
                            ps_t = psp.tile([128, 256], BF16, tag="pt", bufs=1)
                            for j in range(n // 128):
                                nc.tensor.transpose(
                                    ps_t[:, 64 * j:64 * j + 64],
                                    stateT[hr, sco + 128 * j:sco + 128 * j + 128],
                                    w["i64d"][hr, :])
                            nn = sbp.tile([128, 256], BF16, tag="nn")
                            nc.vector.tensor_copy(nn[:, 0:64 * (n // 128)], ps_t[:, 0:64 * (n // 128)])
                            for j in range(n // 128):
                                nc.tensor.matmul(
                                    ps_agg[:],
                                    sst[:, n + 128 * j:n + 128 * j + 128],
                                    nn[:, 64 * j:64 * j + 64],
                                    start=(ci == 0 and j == 0),
                                    stop=(ci == nchunk - 1 and j == n // 128 - 1))
                        agnb = sbp.tile([128, 64], BF16, tag="agnb")
                        nc.scalar.activation(agnb[:], ps_agg[:], AFT.Identity,
                                             scale=invcnt_sb[:, b:b + 1])
                        ps_at = psp.tile([128, 128], BF16, tag="pat", bufs=1)
                        hr2 = slice(64, 128) if b % 2 else slice(0, 64)
                        nc.tensor.transpose(ps_at[hr2, :], agnb[:], w["i128b"][:])
                        nc.vector.tensor_copy(aggT[hr2, (b // 2) * 128:(b // 2) * 128 + 128],
                                              ps_at[hr2, :])
                # node MLP + next-layer tables
                with tc.tile_pool(name=f"sbn{l}", bufs=3) as sbp, \
                     tc.tile_pool(name=f"psn{l}", bufs=2, space="PSUM") as psp:
                    for t in range(NBLK):
                        na = min(128, ASH - 128 * t)
                        hr2 = slice(64, 128) if t % 2 else slice(0, 64)
                        ps_hn = psp.tile([128, 128], F32, tag="hn")
                        nc.tensor.matmul(ps_hn[:, 0:na],
                                         w["nw1a_all"][:, l * HID:(l + 1) * HID],
                                         nodeTb[:, 128 * t:128 * t + na],
                                         start=True, stop=False)
                        nc.tensor.matmul(ps_hn[:, 0:na],
                                         w["nw1b_dup"][hr2, l * HID:(l + 1) * HID],
                                         aggT[hr2, (t // 2) * 128:(t // 2) * 128 + na],
                                         start=False, stop=True)
                        hn = sbp.tile([128, 128], BF16, tag="hn_s")
                        nc.scalar.activation(hn[:, 0:na], ps_hn[:, 0:na], act,
                                             bias=w["nb1"][:, l:l + 1])
                        ps_nd = psp.tile([128, 128], F32, tag="ndl")
                        nc.tensor.matmul(ps_nd[:, 0:na],
                                         w["nw2_all"][:, l * ND:(l + 1) * ND],
                                         hn[:, 0:na], start=True, stop=False)
                        nc.tensor.matmul(ps_nd[:, 0:na], w["i128f"][:],
                                         nodeT[:, 128 * t:128 * t + na], start=False, stop=True)
                        nc.scalar.activation(nodeT[:, 128 * t:128 * t + na], ps_nd[:, 0:na],
                                             AFT.Identity, bias=w["nb2"][:, l:l + 1])
                        nc.vector.tensor_copy(nodeTb[:, 128 * t:128 * t + na],
                                              nodeT[:, 128 * t:128 * t + na])
                    if l < L - 1:
                        node_tables(l + 1, sbp, psp)

            # ---- pooling ----
            with tc.tile_pool(name="sbt", bufs=3) as sbt, \
                 tc.tile_pool(name="pst", bufs=2, space="PSUM") as pst:
                nnat_all = pp.tile([128, NBLK * 128], BF16, tag="nnat_all")
                for t in range(NBLK):
                    na = min(128, ASH - 128 * t)
                    ps_tr = pst.tile([128, 128], F32, tag="ptr")
                    nc.tensor.transpose(ps_tr[0:na, :], nodeT[:, 128 * t:128 * t + na],
                                        w["i128f"][:])
                    nc.vector.tensor_copy(nnat_all[0:na, 128 * t:128 * t + 128][:, 0:128],
                                          ps_tr[0:na, :])
            with tc.tile_pool(name="sbp", bufs=3) as sbp, \
                 tc.tile_pool(name="psp", bufs=1, space="PSUM") as psp:
                pools = [psp.tile([128, 128], F32, tag=f"pool{g}", name=f"pool{g}") for g in range(NG)]
                for t in range(NBLK):
                    na = min(128, ASH - 128 * t)
                    pmt = sbp.tile([128, NG * GC], BF16, tag="pmt")
                    nc.sync.dma_start(pmt[0:na, :], pmat[0:na, t * NG * GC:(t + 1) * NG * GC])
                    for g in range(NG):
                        gc = min(GC, NCRYS - g * GC)
                        nc.tensor.matmul(pools[g][0:gc, :], pmt[0:na, g * GC:g * GC + gc],
                                         nnat_all[0:na, 128 * t:128 * t + 128][:, 0:128],
                                         start=(t == 0), stop=(t == NBLK - 1))
                for g in range(NG):
                    gc = min(GC, NCRYS - g * GC)
                    pev = sbp.tile([128, 128], F32, tag="pev")
                    nc.vector.tensor_copy(pev[0:gc, :], pools[g][0:gc, :])
                    nc.sync.dma_start(pool_in[g * GC:g * GC + gc, :], pev[0:gc, :])
                nc.gpsimd.collective_compute(
                    "AllReduce", mybir.AluOpType.add,
                    replica_groups=[list(range(NCORES))],
                    ins=[pool_in[:].opt()], outs=[pool_out[:].opt()],
                )

            # ---- readout (replicated) ----
            with tc.tile_pool(name="sbr", bufs=2) as sbp, \
                 tc.tile_pool(name="psr", bufs=2, space="PSUM") as psp:
                for g in range(NG):
                    gc = min(GC, NCRYS - g * GC)
                    pg = sbp.tile([128, 128], F32, tag="pg")
                    nc.sync.dma_start(pg[0:gc, :], pool_out[g * GC:g * GC + gc, :])
                    mean = sbp.tile([128, 128], BF16, tag="mean")
                    nc.scalar.activation(mean[0:gc, :], pg[0:gc, :], AFT.Identity,
                                         scale=w["invccnt"][0:gc, g:g + 1])
                    ps_mt = psp.tile([128, 128], BF16, tag="pmt2")
                    nc.tensor.transpose(ps_mt[:, 0:gc], mean[0:gc, :], w["i128b"][0:gc, 0:gc])
                    meanT = sbp.tile([128, 128], BF16, tag="meanT")
                    nc.vector.tensor_copy(meanT[:, 0:gc], ps_mt[:, 0:gc])
                    ps_hr = psp.tile([128, 128], F32, tag="phr")
                    nc.tensor.matmul(ps_hr[:, 0:gc], w["readW"][:], meanT[:, 0:gc],
                                     start=True, stop=True)
                    hrT = sbp.tile([128, 128], BF16, tag="hrT")
                    nc.scalar.activation(hrT[:, 0:gc], ps_hr[:, 0:gc], act,
                                         bias=w["readb"][:])
                    ps_y = psp.tile([128, 128], F32, tag="py")
                    nc.tensor.matmul(ps_y[0:1, 0:gc], w["outW"][:], hrT[:, 0:gc],
                                     start=True, stop=True)
                    ysb = sbp.tile([1, 128], F32, tag="ysb")
                    nc.scalar.activation(ysb[0:1, 0:gc], ps_y[0:1, 0:gc], AFT.Copy,
                                         bias=meta["out_b"])
                    nc.sync.dma_start(y[0:1, g * GC:g * GC + gc], ysb[0:1, 0:gc])

    nc.compile()
    return nc


def run_cores(meta, in_maps, act=AFT.Silu, sim=False):
    nc = _build(meta, act=act)
    if sim:
        from concourse.bass_interp import MultiCoreSim
        s = MultiCoreSim(nc, NCORES, trace=False)
        for k in range(NCORES):
            for nm, arr in in_maps[k].items():
                s.cores[k].tensor(nm)[:] = arr
        s.simulate(check_with_hw=False)
        return [{"y": np.array(s.cores[k].tensor("y"))} for k in range(NCORES)], None
    from concourse import bass_utils
    res = bass_utils.run_bass_kernel_spmd(nc, in_maps, core_ids=list(range(NCORES)))
    return res.results, res


def kernel(**inputs):
    cfg = dict(FULL_CFG)
    n, m = np.asarray(inputs["nbr_fea_idx"]).shape
    cfg["N"], cfg["M"] = int(n), int(m)
    cfg["AFD"] = int(np.asarray(inputs["atom_fea"]).shape[1])
    cfg["EFD"] = int(np.asarray(inputs["nbr_fea"]).shape[2])
    cfg["NCRYS"] = int(inputs["num_crystals"])
    cfg["L"] = int(np.asarray(inputs["eW1"]).shape[0])
    meta, in_maps = _prep(inputs, cfg)
    results, _ = run_cores(meta, in_maps)
    return np.asarray(results[0]["y"], np.float32).reshape(cfg["NCRYS"], 1)



# revision 15
# speedup vs baseline: 3.3392x; 3.3392x over previous
"""CrystalGraphALIGNN Trainium2 kernel (8 NeuronCores, SPMD).

Strategy: dst-shard edges across cores (atom v owned by core v // (N/8); edge
(i,j) owned by the core of its dst). Per core, edges are sorted by dst and
grouped into 128-atom blocks so that:
  - the dst-side expansion A_dst[dst(e)] is a block-local one-hot matmul
    (S^T streamed from DRAM),
  - the scatter-mean aggregation is a one-hot matmul into PSUM (S streamed),
  - only the src side needs a true random gather: per-edge rows of
    A_src = node @ W_src, fetched with dma_gather (transposed, bf16) from a
    DRAM table that is refreshed once per layer via AllGather.
Node states and the node MLP stay fully shard-local; crystal pooling is a
one-hot matmul + a single AllReduce, readout replicated on every core.
"""

import numpy as np
import ml_dtypes

import concourse.bass as bass
import concourse.bacc as bacc
import concourse.mybir as mybir
import concourse.tile as tile
from concourse import library_config

F32 = mybir.dt.float32
BF16 = mybir.dt.bfloat16
I16 = mybir.dt.int16
AFT = mybir.ActivationFunctionType
BF = ml_dtypes.bfloat16

NCORES = 8
ED, ND, HID, RD = 64, 128, 128, 128
EDGE_THRESH = 1e-6
GC = 125  # crystals per pooling group

FULL_CFG = dict(N=50000, M=12, AFD=92, EFD=41, NCRYS=1000, L=4)


def _cdiv(a, b):
    return (a + b - 1) // b


def _wrap_idx(flat):
    """int16 flat idx -> [16, len/16] wrapped layout (replicated to 128 on device)."""
    n = len(flat)
    assert n % 16 == 0
    return flat.reshape(n // 16, 16).T.astype(np.int16)


def _prep(inputs, cfg):
    N, M, AFD, EFD, NCRYS, L = (cfg[k] for k in ("N", "M", "AFD", "EFD", "NCRYS", "L"))
    ASH = N // NCORES
    NBLK = _cdiv(ASH, 128)
    LOS = min(25000, N)  # src index split for int16 gather indices
    NG = _cdiv(NCRYS, GC)

    af = np.asarray(inputs["atom_fea"], np.float32)
    nf = np.asarray(inputs["nbr_fea"], np.float32)
    nidx = np.asarray(inputs["nbr_fea_idx"]).astype(np.int64)
    cb = np.asarray(inputs["crystal_batch"]).astype(np.int64)

    E = N * M
    dst = np.clip(nidx.reshape(-1), 0, N - 1)
    src = np.repeat(np.arange(N, dtype=np.int64), M)
    ea = nf.reshape(E, EFD)
    mask = (np.abs(ea).sum(1) > EDGE_THRESH).astype(np.float32)

    cnt = np.bincount(dst, weights=mask, minlength=N)
    invcnt = (1.0 / np.maximum(cnt, 1.0)).astype(np.float32)
    ccnt = np.bincount(cb, minlength=NCRYS).astype(np.float32)
    invccnt = (1.0 / np.maximum(ccnt, 1.0)).astype(np.float32)

    core_of = dst // ASH
    dloc = dst - core_of * ASH
    blk_of = dloc // 128

    # per-core, per-block, lo/hi edge id lists
    lists = [[[None, None] for _ in range(NBLK)] for _ in range(NCORES)]
    order = np.lexsort((dst, blk_of + core_of * NBLK))  # group by (core, blk)
    for k in range(NCORES):
        esel = order[(core_of[order] == k)]
        for b in range(NBLK):
            eb = esel[blk_of[esel] == b]
            lists[k][b][0] = eb[src[eb] < LOS]
            lists[k][b][1] = eb[src[eb] >= LOS]

    T_lo = np.zeros(NBLK, np.int64)
    T_hi = np.zeros(NBLK, np.int64)
    for b in range(NBLK):
        T_lo[b] = max(_cdiv(max(len(lists[k][b][0]) for k in range(NCORES)), 128), 1)
        T_hi[b] = _cdiv(max(len(lists[k][b][1]) for k in range(NCORES)), 128)
        if (T_lo[b] + T_hi[b]) % 2:
            if N > LOS:
                T_hi[b] += 1
            else:
                T_lo[b] += 1

    # geometry: edge-col space (block-major), state-col space (per half),
    # chunk list entries: (b, half, state_col, edge_col, blk_edge_col, n)
    BHALF = NBLK // 2
    ecol = np.zeros(NBLK + 1, np.int64)
    for b in range(NBLK):
        ecol[b + 1] = ecol[b] + (T_lo[b] + T_hi[b]) * 128
    EP = int(ecol[NBLK])
    scol = np.zeros(NBLK, np.int64)
    acc = [0, 0]
    blocks = []
    for b in range(NBLK):
        half = 0 if b < BHALF else 1
        scol[b] = acc[half]
        nb_e = (T_lo[b] + T_hi[b]) * 128
        acc[half] += nb_e
        tiles = (T_lo[b] + T_hi[b])
        chunks = []
        off = 0
        while tiles > 0:
            t = 4 if tiles >= 4 else tiles
            chunks.append((int(scol[b] + off), int(ecol[b] + off), off, t * 128))
            off += t * 128
            tiles -= t
        blocks.append(dict(b=b, half=half, nblk_e=nb_e, chunks=chunks,
                           n_lo=int(T_lo[b] * 128), n_hi=int(T_hi[b] * 128)))
    EPC = max(acc)
    IWL = sum(int(t) * 8 for t in T_lo)
    IWH = sum(int(t) * 8 for t in T_hi)

    meta = dict(cfg=cfg, ASH=ASH, NBLK=NBLK, LOS=LOS, NG=NG, EP=EP, EPC=EPC,
                BHALF=BHALF, blocks=blocks, IWL=IWL, IWH=IWH, ETILES=EP // 128,
                out_b=float(np.asarray(inputs["out_b"]).reshape(-1)[0]))

    # shared weights
    eW1 = np.asarray(inputs["eW1"], np.float32)
    eW2 = np.asarray(inputs["eW2"], np.float32)
    nW1 = np.asarray(inputs["nW1"], np.float32)
    nW2 = np.asarray(inputs["nW2"], np.float32)

    def bfc(x):
        return np.ascontiguousarray(x, np.float32).astype(BF)

    atomW93 = np.zeros((AFD + 1, ND), np.float32)
    atomW93[:AFD] = np.asarray(inputs["atom_W"], np.float32)
    atomW93[AFD] = np.asarray(inputs["atom_b"], np.float32)
    edgeW42 = np.zeros((EFD + 1, ED), np.float32)
    edgeW42[:EFD] = np.asarray(inputs["edge_W"], np.float32)
    edgeW42[EFD] = np.asarray(inputs["edge_b"], np.float32)

    we_dup = np.zeros((128, L * HID), np.float32)
    nw1b_dup = np.zeros((128, L * ND), np.float32)
    for l in range(L):
        we_dup[0:64, l * HID:(l + 1) * HID] = eW1[l, 0:ED]
        we_dup[64:128, l * HID:(l + 1) * HID] = eW1[l, 0:ED]
        nw1b_dup[0:64, l * ND:(l + 1) * ND] = nW1[l, ND:ND + ED]
        nw1b_dup[64:128, l * ND:(l + 1) * ND] = nW1[l, ND:ND + ED]
    ws_all = np.concatenate([eW1[l, ED:ED + ND] for l in range(L)], 1)      # [128, L*128]
    wd_all = np.concatenate([eW1[l, ED + ND:] for l in range(L)], 1)        # [128, L*128]
    ew2_all = np.concatenate([eW2[l] for l in range(L)], 1)                 # [128, L*64]
    nw1a_all = np.concatenate([nW1[l, 0:ND] for l in range(L)], 1)          # [128, L*128]
    nw2_all = np.concatenate([nW2[l] for l in range(L)], 1)                 # [128, L*128]

    eb1 = np.asarray(inputs["eb1"], np.float32).T.copy()                    # [128, L]
    eb2p = np.zeros((128, L), np.float32)
    eb2p[0:64] = np.asarray(inputs["eb2"], np.float32).T
    eb2p[64:128] = eb2p[0:64]
    nb1 = np.asarray(inputs["nb1"], np.float32).T.copy()
    nb2 = np.asarray(inputs["nb2"], np.float32).T.copy()

    i64d = np.zeros((128, 64), np.float32)
    i64d[0:64] = np.eye(64)
    i64d[64:128] = np.eye(64)

    shared = {
        "atomW": bfc(atomW93), "edgeW": bfc(edgeW42),
        "we_dup": bfc(we_dup), "nw1b_dup": bfc(nw1b_dup),
        "ws_all": bfc(ws_all), "wd_all": bfc(wd_all), "ew2_all": bfc(ew2_all),
        "nw1a_all": bfc(nw1a_all), "nw2_all": bfc(nw2_all),
        "readW": bfc(np.asarray(inputs["read_W"])), "outW": bfc(np.asarray(inputs["out_W"])),
        "eb1": eb1, "eb2p": eb2p, "nb1": nb1, "nb2": nb2,
        "readb": np.asarray(inputs["read_b"], np.float32).reshape(RD, 1),
        "i64d": bfc(i64d), "i128b": bfc(np.eye(128)), "i128f": np.eye(128, dtype=np.float32),
        "invccnt": np.pad(invccnt, (0, NG * GC - NCRYS)).reshape(NG, GC).T.copy(),  # [125, NG]
    }

    in_maps = []
    for k in range(NCORES):
        a0 = k * ASH
        eattrT = np.zeros((EFD + 1, EP), np.float32)
        arow_f = np.full(EP, -1.0, np.float32)   # block-local dst row per edge col
        mask_f = np.zeros(EP, np.float32)
        idxl = np.zeros(sum(int(t) * 128 for t in T_lo), np.int64)
        idxh = np.zeros(sum(int(t) * 128 for t in T_hi), np.int64)
        ol = oh = 0
        for blk in blocks:
            b = blk["b"]
            ids_lo, ids_hi = lists[k][b]
            n_lo, n_hi = blk["n_lo"], blk["n_hi"]
            eo = int(ecol[b])
            ids = np.full(n_lo + n_hi, -1, np.int64)
            ids[:len(ids_lo)] = ids_lo
            ids[n_lo:n_lo + len(ids_hi)] = ids_hi
            real = ids >= 0
            rids = ids[real]
            eattrT[:EFD, eo:eo + n_lo + n_hi][:, real] = ea[rids].T
            eattrT[EFD, eo:eo + n_lo + n_hi][real] = 1.0
            arow_f[eo:eo + n_lo + n_hi][real] = (dloc[rids] - 128 * b).astype(np.float32)
            mask_f[eo:eo + n_lo + n_hi][real] = mask[rids]
            gl = np.zeros(n_lo, np.int64)
            gl[:len(ids_lo)] = src[ids_lo]
            gh = np.zeros(n_hi, np.int64)
            gh[:len(ids_hi)] = src[ids_hi] - LOS
            idxl[ol:ol + n_lo] = gl
            idxh[oh:oh + n_hi] = gh
            ol += n_lo
            oh += n_hi

        inv_sb = np.ones((128, NBLK), np.float32)
        for b in range(NBLK):
            na = min(128, ASH - 128 * b)
            inv_sb[0:na, b] = invcnt[a0 + 128 * b: a0 + 128 * b + na]
        afT = np.zeros((AFD + 1, ASH), np.float32)
        afT[:AFD] = af[a0:a0 + ASH].T
        afT[AFD] = 1.0
        cbl = np.full((128, NBLK), -1.0, np.float32)  # crystal id per atom row
        for b in range(NBLK):
            na = min(128, ASH - 128 * b)
            cbl[0:na, b] = cb[a0 + 128 * b: a0 + 128 * b + na]

        m = {
            "eattrT": eattrT.astype(BF),
            "arow": arow_f.reshape(EP // 128, 128).T.copy(),
            "maskv": mask_f.reshape(EP // 128, 128).T.copy(),
            "idxlo": _wrap_idx(idxl), "invcnt": inv_sb,
            "afT": afT.astype(BF), "cbloc": cbl,
        }
        if IWH:
            m["idxhi"] = _wrap_idx(idxh)
        m.update(shared)
        in_maps.append(m)
    return meta, in_maps


def _build(meta, act=AFT.Silu, noop=False, no_gather=False, no_coll=False):
    cfg = meta["cfg"]
    N, M, AFD, EFD, NCRYS, L = (cfg[k] for k in ("N", "M", "AFD", "EFD", "NCRYS", "L"))
    ASH, NBLK, LOS, NG = meta["ASH"], meta["NBLK"], meta["LOS"], meta["NG"]
    EP, EPC, blocks = meta["EP"], meta["EPC"], meta["blocks"]
    IWL, IWH, ETILES = meta["IWL"], meta["IWH"], meta["ETILES"]

    nc = bacc.Bacc("TRN2", target_bir_lowering=False, debug=False, num_devices=NCORES,
                   num_swdge_queues=4)

    def din(name, shape, dt):
        return nc.dram_tensor(name, shape, dt, kind="ExternalInput")

    eattrT = din("eattrT", [EFD + 1, EP], BF16)
    arowd = din("arow", [128, ETILES], F32)
    maskd = din("maskv", [128, ETILES], F32)
    idxlo = din("idxlo", [16, IWL], I16)
    idxhi = din("idxhi", [16, IWH], I16) if IWH else None
    invcnt = din("invcnt", [128, NBLK], F32)
    afT = din("afT", [AFD + 1, ASH], BF16)
    cblocd = din("cbloc", [128, NBLK], F32)
    wts = {}
    for nm, sh, dt in [
        ("atomW", [AFD + 1, ND], BF16), ("edgeW", [EFD + 1, ED], BF16),
        ("we_dup", [128, L * HID], BF16), ("nw1b_dup", [128, L * ND], BF16),
        ("ws_all", [ND, L * HID], BF16), ("wd_all", [ND, L * HID], BF16),
        ("ew2_all", [HID, L * ED], BF16), ("nw1a_all", [ND, L * HID], BF16),
        ("nw2_all", [HID, L * ND], BF16), ("readW", [ND, RD], BF16),
        ("outW", [RD, 1], BF16), ("eb1", [128, L], F32), ("eb2p", [128, L], F32),
        ("nb1", [128, L], F32), ("nb2", [128, L], F32), ("readb", [RD, 1], F32),
        ("i64d", [128, 64], BF16), ("i128b", [128, 128], BF16),
        ("i128f", [128, 128], F32), ("invccnt", [GC, NG], F32),
    ]:
        wts[nm] = din(nm, sh, dt)
    y = nc.dram_tensor("y", [1, NCRYS], F32, kind="ExternalOutput")

    if noop:
        with tile.TileContext(nc) as tc:
            with tc.tile_pool(name="sbz", bufs=1) as sbz:
                yz = sbz.tile([1, NCRYS], F32, tag="yz")
                nc.gpsimd.memset(yz[:], 0.0)
                nc.sync.dma_start(y[:], yz[:])
        nc.compile()
        return nc

    with tile.TileContext(nc) as tc:
        with (
            tc.tile_pool(name="persist", bufs=1) as pp,
            tc.tile_pool(name="dram", bufs=1, space="DRAM") as dp,
        ):
            nc.gpsimd.load_library(library_config.mlp)
            w = {nm: pp.tile(t.shape, t.dtype, tag=nm, name=f"w_{nm}") for nm, t in wts.items()}
            for nm, t in wts.items():
                nc.sync.dma_start(w[nm][:], t[:])
            invcnt_sb = pp.tile([128, NBLK], F32, tag="invcnt_sb")
            nc.sync.dma_start(invcnt_sb[:], invcnt[:])
            stateT = pp.tile([128, EPC], BF16, tag="stateT")
            nodeT = pp.tile([128, ASH], F32, tag="nodeT")
            nodeTb = pp.tile([128, ASH], BF16, tag="nodeTb")
            adst = pp.tile([128, NBLK * 128], BF16, tag="adst")
            aggT = pp.tile([128, _cdiv(NBLK, 2) * 128], BF16, tag="aggT")
            idxsb = pp.tile([128, IWL], I16, tag="idxsb")
            for r in range(8):
                nc.sync.dma_start(idxsb[16 * r:16 * r + 16, :], idxlo[:])
            if IWH:
                idxsbh = pp.tile([128, IWH], I16, tag="idxsbh")
                for r in range(8):
                    nc.sync.dma_start(idxsbh[16 * r:16 * r + 16, :], idxhi[:])
            ssd = dp.tile([128, 2 * EP], BF16, name="ssd", tag="ssd")
            asrc_in = dp.tile([ASH, ND], BF16)
            asrc_fulls = [dp.tile([N, ND], BF16, addr_space="Shared", name=f"asrc_full{i}", tag=f"asrc_full{i}")
                          for i in range(L)]
            pool_in = dp.tile([NCRYS, ND], F32)
            pool_out = dp.tile([NCRYS, ND], F32, addr_space="Shared")

            def node_tables(lw, sbp, psp):
                """A_src shard -> bounce -> AllGather; A_dst blocks (layer lw)."""
                for t in range(NBLK):
                    na = min(128, ASH - 128 * t)
                    lhs = nodeTb[:, 128 * t:128 * t + na]
                    ps_s = psp.tile([128, 128], F32, tag="ps_s")
                    nc.tensor.matmul(ps_s[0:na, :], lhs, w["ws_all"][:, lw * HID:(lw + 1) * HID],
                                     start=True, stop=True)
                    asb = sbp.tile([128, 128], BF16, tag="asb")
                    nc.vector.tensor_copy(asb[0:na, :], ps_s[0:na, :])
                    nc.sync.dma_start(asrc_in[128 * t:128 * t + na, :], asb[0:na, :])
                    ps_d = psp.tile([128, 128], F32, tag="ps_d")
                    nc.tensor.matmul(ps_d[0:na, :], lhs, w["wd_all"][:, lw * HID:(lw + 1) * HID],
                                     start=True, stop=True)
                    nc.vector.tensor_copy(adst[0:na, 128 * t:128 * t + 128][:, 0:128],
                                          ps_d[0:na, :])
                if not no_coll:
                    nc.gpsimd.collective_compute(
                        "AllGather", mybir.AluOpType.bypass,
                        replica_groups=[list(range(NCORES))],
                        ins=[asrc_in[:].opt()], outs=[asrc_fulls[lw][:].opt()],
                    )

            # ---- init: projections + layer-0 tables ----
            with tc.tile_pool(name="sbi", bufs=3) as sbp, \
                 tc.tile_pool(name="psi", bufs=2, space="PSUM") as psp:
                for t in range(NBLK):
                    na = min(128, ASH - 128 * t)
                    aft = sbp.tile([AFD + 1, 128], BF16, tag="aft")
                    nc.sync.dma_start(aft[:, 0:na], afT[:, 128 * t:128 * t + na])
                    ps_n = psp.tile([128, 128], F32, tag="ps_n")
                    nc.tensor.matmul(ps_n[:, 0:na], w["atomW"][:], aft[:, 0:na],
                                     start=True, stop=True)
                    nc.vector.tensor_copy(nodeT[:, 128 * t:128 * t + na], ps_n[:, 0:na])
                    nc.vector.tensor_copy(nodeTb[:, 128 * t:128 * t + na], ps_n[:, 0:na])
                node_tables(0, sbp, psp)
            with tc.tile_pool(name="sbi2", bufs=3) as sbp, \
                 tc.tile_pool(name="psi2", bufs=2, space="PSUM") as psp:
                arow_sb = sbp.tile([128, ETILES], F32, tag="arow_sb", bufs=1)
                nc.sync.dma_start(arow_sb[:], arowd[:])
                mask_sb = sbp.tile([128, ETILES], F32, tag="mask_sb", bufs=1)
                nc.sync.dma_start(mask_sb[:], maskd[:])
                ramp128 = sbp.tile([128, 128], F32, tag="ramp128", bufs=1)
                nc.gpsimd.iota(ramp128[:], pattern=[[1, 128]], base=0,
                               channel_multiplier=0,
                               allow_small_or_imprecise_dtypes=True)
                for blk in blocks:
                    hr = slice(64, 128) if blk["half"] else slice(0, 64)
                    for (sco, eco, bco, n) in blk["chunks"]:
                        eat = sbp.tile([EFD + 1, 512], BF16, tag="eat")
                        nc.sync.dma_start(eat[:, 0:n], eattrT[:, eco:eco + n])
                        ps_e = psp.tile([128, 512], F32, tag="ps_e")
                        nc.tensor.matmul(ps_e[hr, 0:n], w["edgeW"][:], eat[:, 0:n],
                                         start=True, stop=True)
                        nc.vector.tensor_copy(stateT[hr, sco:sco + n], ps_e[hr, 0:n])
                        # build scatter one-hots for this chunk into DRAM ssd:
                        # S^T [atom, edge] (unmasked) and S tiles [edge, atom] (masked)
                        sb_s = sbp.tile([128, 512], BF16, tag="sb_s")
                        ps_tr = psp.tile([128, 512], BF16, tag="ps_tr")
                        sb_st = sbp.tile([128, 512], BF16, tag="sb_st")
                        for j in range(n // 128):
                            kk = eco // 128 + j
                            oh = sbp.tile([128, 128], BF16, tag="oh")
                            nc.vector.tensor_scalar(
                                oh[:], ramp128[:], arow_sb[:, kk:kk + 1], None,
                                mybir.AluOpType.is_equal)
                            nc.vector.tensor_scalar(
                                sb_s[:, 128 * j:128 * j + 128], ramp128[:],
                                arow_sb[:, kk:kk + 1], mask_sb[:, kk:kk + 1],
                                mybir.AluOpType.is_equal, mybir.AluOpType.mult)
                            nc.tensor.transpose(ps_tr[:, 128 * j:128 * j + 128],
                                                oh[:], w["i128b"][:])
                        nc.vector.tensor_copy(sb_st[:, 0:n], ps_tr[:, 0:n])
                        nc.sync.dma_start(ssd[:, 2 * eco:2 * eco + n], sb_st[:, 0:n])
                        nc.sync.dma_start(ssd[:, 2 * eco + n:2 * eco + 2 * n],
                                          sb_s[:, 0:n])

            # ---- layers ----
            for l in range(L):
                with tc.tile_pool(name=f"sbe{l}", bufs=3) as sbp, \
                     tc.tile_pool(name=f"pse{l}", bufs=2, space="PSUM") as psp, \
                     tc.tile_pool(name=f"psg{l}", bufs=2, space="PSUM") as psg:
                    for blk in blocks:
                        b = blk["b"]
                        hr = slice(64, 128) if blk["half"] else slice(0, 64)
                        ba = min(128, ASH - 128 * b)
                        asrc_full = asrc_fulls[l]
                        gt = sbp.tile([128, 1, blk["nblk_e"]], BF16, tag="gt", bufs=2)
                        if blk["n_lo"] and not no_gather:
                            io = sum(bb["n_lo"] for bb in blocks[:b]) // 16
                            nc.gpsimd.dma_gather(
                                gt[:, :, 0:blk["n_lo"]], asrc_full[0:LOS, :],
                                idxsb[:, io:io + blk["n_lo"] // 16],
                                blk["n_lo"], blk["n_lo"], ND, transpose=True,
                                queue_num=(2 * b) % 4)
                        if blk["n_hi"] and not no_gather:
                            io = sum(bb["n_hi"] for bb in blocks[:b]) // 16
                            nc.gpsimd.dma_gather(
                                gt[:, :, blk["n_lo"]:], asrc_full[LOS:N, :],
                                idxsbh[:, io:io + blk["n_hi"] // 16],
                                blk["n_hi"], blk["n_hi"], ND, transpose=True,
                                queue_num=(2 * b + 1) % 4)
                        ps_agg = psg.tile([128, 64], F32, tag="agg")
                        nchunk = len(blk["chunks"])
                        e0 = blk["chunks"][0][1]
                        ssb = sbp.tile([128, 2 * blk["nblk_e"]], BF16, tag="ssb", bufs=2)
                        nc.sync.dma_start(ssb[:, 0:2 * blk["nblk_e"]],
                                          ssd[:, 2 * e0:2 * e0 + 2 * blk["nblk_e"]])
                        for ci, (sco, eco, bco, n) in enumerate(blk["chunks"]):
                            sst = ssb[:, 2 * (eco - e0):2 * (eco - e0) + 2 * n]
                            ps_h = psp.tile([128, 512], F32, tag="ph")
                            nc.tensor.matmul(ps_h[:, 0:n], adst[0:ba, 128 * b:128 * b + 128],
                                             sst[0:ba, 0:n], start=True, stop=False)  # S^T chunk
                            nc.tensor.matmul(ps_h[:, 0:n], w["we_dup"][hr, l * HID:(l + 1) * HID],
                                             stateT[hr, sco:sco + n], start=False,
                                             stop=no_gather)
                            if not no_gather:
                                nc.tensor.matmul(ps_h[:, 0:n], w["i128b"][:],
                                                 gt[:, 0, bco:bco + n], start=False, stop=True)
                            ht = sbp.tile([128, 512], BF16, tag="ht")
                            nc.scalar.activation(ht[:, 0:n], ps_h[:, 0:n], act,
                                                 bias=w["eb1"][:, l:l + 1])
                            ps_dd = psp.tile([128, 512], F32, tag="pd")
                            nc.tensor.matmul(ps_dd[hr, 0:n], w["i64d"][hr, :],
                                             stateT[hr, sco:sco + n], start=True, stop=False)
                            nc.tensor.matmul(ps_dd[hr, 0:n], w["ew2_all"][:, l * ED:(l + 1) * ED],
                                             ht[:, 0:n], start=False, stop=True)
                            nc.vector.tensor_scalar(stateT[hr, sco:sco + n],
                                                    ps_dd[hr, 0:n],
                                                    w["eb2p"][hr, l:l + 1], None,
                                                    mybir.AluOpType.add)
                            ps_t = psp.tile([128, 256], BF16, tag="pt", bufs=1)
                            for j in range(n // 128):
                                nc.tensor.transpose(
                                    ps_t[:, 64 * j:64 * j + 64],
                                    stateT[hr, sco + 128 * j:sco + 128 * j + 128],
                                    w["i64d"][hr, :])
                            nn = sbp.tile([128, 256], BF16, tag="nn")
                            nc.vector.tensor_copy(nn[:, 0:64 * (n // 128)], ps_t[:, 0:64 * (n // 128)])
                            for j in range(n // 128):
                                nc.tensor.matmul(
                                    ps_agg[:],
                                    sst[:, n + 128 * j:n + 128 * j + 128],
                                    nn[:, 64 * j:64 * j + 64],
                                    start=(ci == 0 and j == 0),
                                    stop=(ci == nchunk - 1 and j == n // 128 - 1))
                        agnb = sbp.tile([128, 64], BF16, tag="agnb")
                        nc.scalar.activation(agnb[:], ps_agg[:], AFT.Identity,
                                             scale=invcnt_sb[:, b:b + 1])
                        ps_at = psp.tile([128, 128], BF16, tag="pat", bufs=1)
                        hr2 = slice(64, 128) if b % 2 else slice(0, 64)
                        nc.tensor.transpose(ps_at[hr2, :], agnb[:], w["i128b"][:])
                        nc.vector.tensor_copy(aggT[hr2, (b // 2) * 128:(b // 2) * 128 + 128],
                                              ps_at[hr2, :])
                # node MLP + next-layer tables
                with tc.tile_pool(name=f"sbn{l}", bufs=3) as sbp, \
                     tc.tile_pool(name=f"psn{l}", bufs=2, space="PSUM") as psp:
                    for t in range(NBLK):
                        na = min(128, ASH - 128 * t)
                        hr2 = slice(64, 128) if t % 2 else slice(0, 64)
                        ps_hn = psp.tile([128, 128], F32, tag="hn")
                        nc.tensor.matmul(ps_hn[:, 0:na],
                                         w["nw1a_all"][:, l * HID:(l + 1) * HID],
                                         nodeTb[:, 128 * t:128 * t + na],
                                         start=True, stop=False)
                        nc.tensor.matmul(ps_hn[:, 0:na],
                                         w["nw1b_dup"][hr2, l * HID:(l + 1) * HID],
                                         aggT[hr2, (t // 2) * 128:(t // 2) * 128 + na],
                                         start=False, stop=True)
                        hn = sbp.tile([128, 128], BF16, tag="hn_s")
                        nc.scalar.activation(hn[:, 0:na], ps_hn[:, 0:na], act,
                                             bias=w["nb1"][:, l:l + 1])
                        ps_nd = psp.tile([128, 128], F32, tag="ndl")
                        nc.tensor.matmul(ps_nd[:, 0:na],
                                         w["nw2_all"][:, l * ND:(l + 1) * ND],
                                         hn[:, 0:na], start=True, stop=False)
                        nc.tensor.matmul(ps_nd[:, 0:na], w["i128f"][:],
                                         nodeT[:, 128 * t:128 * t + na], start=False, stop=True)
                        nc.scalar.activation(nodeT[:, 128 * t:128 * t + na], ps_nd[:, 0:na],
                                             AFT.Identity, bias=w["nb2"][:, l:l + 1])
                        nc.vector.tensor_copy(nodeTb[:, 128 * t:128 * t + na],
                                              nodeT[:, 128 * t:128 * t + na])
                    if l < L - 1:
                        node_tables(l + 1, sbp, psp)

            # ---- pooling ----
            with tc.tile_pool(name="sbt", bufs=3) as sbt, \
                 tc.tile_pool(name="pst", bufs=2, space="PSUM") as pst:
                nnat_all = pp.tile([128, NBLK * 128], BF16, tag="nnat_all")
                for t in range(NBLK):
                    na = min(128, ASH - 128 * t)
                    ps_tr = pst.tile([128, 128], F32, tag="ptr")
                    nc.tensor.transpose(ps_tr[0:na, :], nodeT[:, 128 * t:128 * t + na],
                                        w["i128f"][:])
                    nc.vector.tensor_copy(nnat_all[0:na, 128 * t:128 * t + 128][:, 0:128],
                                          ps_tr[0:na, :])
            with tc.tile_pool(name="sbp", bufs=3) as sbp, \
                 tc.tile_pool(name="psp", bufs=1, space="PSUM") as psp:
                pools = [psp.tile([128, 128], F32, tag=f"pool{g}", name=f"pool{g}") for g in range(NG)]
                rampP = sbp.tile([128, NG * GC], F32, tag="rampP", bufs=1)
                nc.gpsimd.iota(rampP[:], pattern=[[1, NG * GC]], base=0,
                               channel_multiplier=0,
                               allow_small_or_imprecise_dtypes=True)
                cbloc_sb = sbp.tile([128, NBLK], F32, tag="cbloc_sb", bufs=1)
                nc.sync.dma_start(cbloc_sb[:], cblocd[:])
                for t in range(NBLK):
                    na = min(128, ASH - 128 * t)
                    pmt = sbp.tile([128, NG * GC], BF16, tag="pmt")
                    nc.vector.tensor_scalar(pmt[:], rampP[:], cbloc_sb[:, t:t + 1],
                                            None, mybir.AluOpType.is_equal)
                    for g in range(NG):
                        gc = min(GC, NCRYS - g * GC)
                        nc.tensor.matmul(pools[g][0:gc, :], pmt[0:na, g * GC:g * GC + gc],
                                         nnat_all[0:na, 128 * t:128 * t + 128][:, 0:128],
                                         start=(t == 0), stop=(t == NBLK - 1))
                for g in range(NG):
                    gc = min(GC, NCRYS - g * GC)
                    pev = sbp.tile([128, 128], F32, tag="pev")
                    nc.vector.tensor_copy(pev[0:gc, :], pools[g][0:gc, :])
                    nc.sync.dma_start(pool_in[g * GC:g * GC + gc, :], pev[0:gc, :])
                nc.gpsimd.collective_compute(
                    "AllReduce", mybir.AluOpType.add,
                    replica_groups=[list(range(NCORES))],
                    ins=[pool_in[:].opt()], outs=[pool_out[:].opt()],
                )

            # ---- readout (replicated) ----
            with tc.tile_pool(name="sbr", bufs=2) as sbp, \
                 tc.tile_pool(name="psr", bufs=2, space="PSUM") as psp:
                for g in range(NG):
                    gc = min(GC, NCRYS - g * GC)
                    pg = sbp.tile([128, 128], F32, tag="pg")
                    nc.sync.dma_start(pg[0:gc, :], pool_out[g * GC:g * GC + gc, :])
                    mean = sbp.tile([128, 128], BF16, tag="mean")
                    nc.scalar.activation(mean[0:gc, :], pg[0:gc, :], AFT.Identity,
                                         scale=w["invccnt"][0:gc, g:g + 1])
                    ps_mt = psp.tile([128, 128], BF16, tag="pmt2")
                    nc.tensor.transpose(ps_mt[:, 0:gc], mean[0:gc, :], w["i128b"][0:gc, 0:gc])
                    meanT = sbp.tile([128, 128], BF16, tag="meanT")
                    nc.vector.tensor_copy(meanT[:, 0:gc], ps_mt[:, 0:gc])
                    ps_hr = psp.tile([128, 128], F32, tag="phr")
                    nc.tensor.matmul(ps_hr[:, 0:gc], w["readW"][:], meanT[:, 0:gc],
                                     start=True, stop=True)
                    hrT = sbp.tile([128, 128], BF16, tag="hrT")
                    nc.scalar.activation(hrT[:, 0:gc], ps_hr[:, 0:gc], act,
                                         bias=w["readb"][:])
                    ps_y = psp.tile([128, 128], F32, tag="py")
                    nc.tensor.matmul(ps_y[0:1, 0:gc], w["outW"][:], hrT[:, 0:gc],
                                     start=True, stop=True)
                    ysb = sbp.tile([1, 128], F32, tag="ysb")
                    nc.scalar.activation(ysb[0:1, 0:gc], ps_y[0:1, 0:gc], AFT.Copy,
                                         bias=meta["out_b"])
                    nc.sync.dma_start(y[0:1, g * GC:g * GC + gc], ysb[0:1, 0:gc])

    nc.compile()
    return nc


def run_cores(meta, in_maps, act=AFT.Silu, sim=False):
    nc = _build(meta, act=act)
    if sim:
        from concourse.bass_interp import MultiCoreSim
        s = MultiCoreSim(nc, NCORES, trace=False)
        for k in range(NCORES):
            for nm, arr in in_maps[k].items():
                s.cores[k].tensor(nm)[:] = arr
        s.simulate(check_with_hw=False)
        return [{"y": np.array(s.cores[k].tensor("y"))} for k in range(NCORES)], None
    from concourse import bass_utils
    res = bass_utils.run_bass_kernel_spmd(nc, in_maps, core_ids=list(range(NCORES)))
    return res.results, res


def kernel(**inputs):
    cfg = dict(FULL_CFG)
    n, m = np.asarray(inputs["nbr_fea_idx"]).shape
    cfg["N"], cfg["M"] = int(n), int(m)
    cfg["AFD"] = int(np.asarray(inputs["atom_fea"]).shape[1])
    cfg["EFD"] = int(np.asarray(inputs["nbr_fea"]).shape[2])
    cfg["NCRYS"] = int(inputs["num_crystals"])
    cfg["L"] = int(np.asarray(inputs["eW1"]).shape[0])
    meta, in_maps = _prep(inputs, cfg)
    results, _ = run_cores(meta, in_maps)
    return np.asarray(results[0]["y"], np.float32).reshape(cfg["NCRYS"], 1)



# revision 18
# speedup vs baseline: 3.5134x; 1.0522x over previous
"""CrystalGraphALIGNN Trainium2 kernel (8 NeuronCores, SPMD).

Strategy: dst-shard edges across cores (atom v owned by core v // (N/8); edge
(i,j) owned by the core of its dst). Per core, edges are sorted by dst and
grouped into 128-atom blocks so that:
  - the dst-side expansion A_dst[dst(e)] is a block-local one-hot matmul
    (S^T streamed from DRAM),
  - the scatter-mean aggregation is a one-hot matmul into PSUM (S streamed),
  - only the src side needs a true random gather: per-edge rows of
    A_src = node @ W_src, fetched with dma_gather (transposed, bf16) from a
    DRAM table that is refreshed once per layer via AllGather.
Node states and the node MLP stay fully shard-local; crystal pooling is a
one-hot matmul + a single AllReduce, readout replicated on every core.
"""

import numpy as np
import ml_dtypes

import concourse.bass as bass
import concourse.bacc as bacc
import concourse.mybir as mybir
import concourse.tile as tile
from concourse import library_config

F32 = mybir.dt.float32
BF16 = mybir.dt.bfloat16
I16 = mybir.dt.int16
AFT = mybir.ActivationFunctionType
BF = ml_dtypes.bfloat16

NCORES = 8
ED, ND, HID, RD = 64, 128, 128, 128
EDGE_THRESH = 1e-6
GC = 125  # crystals per pooling group

FULL_CFG = dict(N=50000, M=12, AFD=92, EFD=41, NCRYS=1000, L=4)


def _cdiv(a, b):
    return (a + b - 1) // b


def _wrap_idx(flat):
    """int16 flat idx -> [16, len/16] wrapped layout (replicated to 128 on device)."""
    n = len(flat)
    assert n % 16 == 0
    return flat.reshape(n // 16, 16).T.astype(np.int16)


def _prep(inputs, cfg):
    N, M, AFD, EFD, NCRYS, L = (cfg[k] for k in ("N", "M", "AFD", "EFD", "NCRYS", "L"))
    ASH = N // NCORES
    NBLK = _cdiv(ASH, 128)
    LOS = min(25000, N)  # src index split for int16 gather indices
    NG = _cdiv(NCRYS, GC)

    af = np.asarray(inputs["atom_fea"], np.float32)
    nf = np.asarray(inputs["nbr_fea"], np.float32)
    nidx = np.asarray(inputs["nbr_fea_idx"]).astype(np.int64)
    cb = np.asarray(inputs["crystal_batch"]).astype(np.int64)

    E = N * M
    dst = np.clip(nidx.reshape(-1), 0, N - 1)
    src = np.repeat(np.arange(N, dtype=np.int64), M)
    ea = nf.reshape(E, EFD)
    mask = (np.abs(ea).sum(1) > EDGE_THRESH).astype(np.float32)

    cnt = np.bincount(dst, weights=mask, minlength=N)
    invcnt = (1.0 / np.maximum(cnt, 1.0)).astype(np.float32)
    ccnt = np.bincount(cb, minlength=NCRYS).astype(np.float32)
    invccnt = (1.0 / np.maximum(ccnt, 1.0)).astype(np.float32)

    core_of = dst // ASH
    dloc = dst - core_of * ASH
    blk_of = dloc // 128

    # per-core, per-block, lo/hi edge id lists
    lists = [[[None, None] for _ in range(NBLK)] for _ in range(NCORES)]
    order = np.lexsort((dst, blk_of + core_of * NBLK))  # group by (core, blk)
    for k in range(NCORES):
        esel = order[(core_of[order] == k)]
        for b in range(NBLK):
            eb = esel[blk_of[esel] == b]
            lists[k][b][0] = eb[src[eb] < LOS]
            lists[k][b][1] = eb[src[eb] >= LOS]

    T_lo = np.zeros(NBLK, np.int64)
    T_hi = np.zeros(NBLK, np.int64)
    for b in range(NBLK):
        T_lo[b] = max(_cdiv(max(len(lists[k][b][0]) for k in range(NCORES)), 128), 1)
        T_hi[b] = _cdiv(max(len(lists[k][b][1]) for k in range(NCORES)), 128)
        if (T_lo[b] + T_hi[b]) % 2:
            if N > LOS:
                T_hi[b] += 1
            else:
                T_lo[b] += 1

    # geometry: edge-col space (block-major), state-col space (per half),
    # chunk list entries: (b, half, state_col, edge_col, blk_edge_col, n)
    BHALF = NBLK // 2
    ecol = np.zeros(NBLK + 1, np.int64)
    for b in range(NBLK):
        ecol[b + 1] = ecol[b] + (T_lo[b] + T_hi[b]) * 128
    EP = int(ecol[NBLK])
    scol = np.zeros(NBLK, np.int64)
    acc = [0, 0]
    blocks = []
    for b in range(NBLK):
        half = 0 if b < BHALF else 1
        scol[b] = acc[half]
        nb_e = (T_lo[b] + T_hi[b]) * 128
        acc[half] += nb_e
        tiles = (T_lo[b] + T_hi[b])
        chunks = []
        off = 0
        while tiles > 0:
            t = 4 if tiles >= 4 else tiles
            chunks.append((int(scol[b] + off), int(ecol[b] + off), off, t * 128))
            off += t * 128
            tiles -= t
        blocks.append(dict(b=b, half=half, nblk_e=nb_e, chunks=chunks,
                           n_lo=int(T_lo[b] * 128), n_hi=int(T_hi[b] * 128)))
    EPC = max(acc)
    IWL = sum(int(t) * 8 for t in T_lo)
    IWH = sum(int(t) * 8 for t in T_hi)

    meta = dict(cfg=cfg, ASH=ASH, NBLK=NBLK, LOS=LOS, NG=NG, EP=EP, EPC=EPC,
                BHALF=BHALF, blocks=blocks, IWL=IWL, IWH=IWH, ETILES=EP // 128,
                out_b=float(np.asarray(inputs["out_b"]).reshape(-1)[0]))

    # shared weights
    eW1 = np.asarray(inputs["eW1"], np.float32)
    eW2 = np.asarray(inputs["eW2"], np.float32)
    nW1 = np.asarray(inputs["nW1"], np.float32)
    nW2 = np.asarray(inputs["nW2"], np.float32)

    def bfc(x):
        return np.ascontiguousarray(x, np.float32).astype(BF)

    # int8 feature shipping: fold the dequant scale into the projection
    # weights; bias row ships as q=127 with weight row bias/127.
    s_af = float(np.abs(af).max()) / 127.0
    s_ea = float(np.abs(ea).max()) / 127.0
    atomW93 = np.zeros((AFD + 1, ND), np.float32)
    atomW93[:AFD] = np.asarray(inputs["atom_W"], np.float32) * s_af
    atomW93[AFD] = np.asarray(inputs["atom_b"], np.float32) / 127.0
    edgeW42 = np.zeros((EFD + 1, ED), np.float32)
    edgeW42[:EFD] = np.asarray(inputs["edge_W"], np.float32) * s_ea
    edgeW42[EFD] = np.asarray(inputs["edge_b"], np.float32) / 127.0

    we_dup = np.zeros((128, L * HID), np.float32)
    nw1b_dup = np.zeros((128, L * ND), np.float32)
    for l in range(L):
        we_dup[0:64, l * HID:(l + 1) * HID] = eW1[l, 0:ED]
        we_dup[64:128, l * HID:(l + 1) * HID] = eW1[l, 0:ED]
        nw1b_dup[0:64, l * ND:(l + 1) * ND] = nW1[l, ND:ND + ED]
        nw1b_dup[64:128, l * ND:(l + 1) * ND] = nW1[l, ND:ND + ED]
    ws_all = np.concatenate([eW1[l, ED:ED + ND] for l in range(L)], 1)      # [128, L*128]
    wd_all = np.concatenate([eW1[l, ED + ND:] for l in range(L)], 1)        # [128, L*128]
    ew2_all = np.concatenate([eW2[l] for l in range(L)], 1)                 # [128, L*64]
    nw1a_all = np.concatenate([nW1[l, 0:ND] for l in range(L)], 1)          # [128, L*128]
    nw2_all = np.concatenate([nW2[l] for l in range(L)], 1)                 # [128, L*128]

    eb1 = np.asarray(inputs["eb1"], np.float32).T.copy()                    # [128, L]
    eb2p = np.zeros((128, L), np.float32)
    eb2p[0:64] = np.asarray(inputs["eb2"], np.float32).T
    eb2p[64:128] = eb2p[0:64]
    nb1 = np.asarray(inputs["nb1"], np.float32).T.copy()
    nb2 = np.asarray(inputs["nb2"], np.float32).T.copy()

    shared = {
        "atomW": bfc(atomW93), "edgeW": bfc(edgeW42),
        "we_dup": bfc(we_dup), "nw1b_dup": bfc(nw1b_dup),
        "ws_all": bfc(ws_all), "wd_all": bfc(wd_all), "ew2_all": bfc(ew2_all),
        "nw1a_all": bfc(nw1a_all), "nw2_all": bfc(nw2_all),
        "readW": bfc(np.asarray(inputs["read_W"])), "outW": bfc(np.asarray(inputs["out_W"])),
        "eb1": eb1, "eb2p": eb2p, "nb1": nb1, "nb2": nb2,
        "readb": np.asarray(inputs["read_b"], np.float32).reshape(RD, 1),
        "invccnt": np.pad(invccnt, (0, NG * GC - NCRYS)).reshape(NG, GC).T.copy(),  # [125, NG]
    }

    in_maps = []
    for k in range(NCORES):
        a0 = k * ASH
        eattrT = np.zeros((EFD + 1, EP), np.float32)
        arow_f = np.full(EP, -1.0, np.float32)   # block-local dst row per edge col
        mask_f = np.zeros(EP, np.float32)
        idxl = np.zeros(sum(int(t) * 128 for t in T_lo), np.int64)
        idxh = np.zeros(sum(int(t) * 128 for t in T_hi), np.int64)
        ol = oh = 0
        for blk in blocks:
            b = blk["b"]
            ids_lo, ids_hi = lists[k][b]
            n_lo, n_hi = blk["n_lo"], blk["n_hi"]
            eo = int(ecol[b])
            ids = np.full(n_lo + n_hi, -1, np.int64)
            ids[:len(ids_lo)] = ids_lo
            ids[n_lo:n_lo + len(ids_hi)] = ids_hi
            real = ids >= 0
            rids = ids[real]
            eattrT[:EFD, eo:eo + n_lo + n_hi][:, real] = np.round(ea[rids].T / s_ea)
            eattrT[EFD, eo:eo + n_lo + n_hi][real] = 127.0
            arow_f[eo:eo + n_lo + n_hi][real] = (dloc[rids] - 128 * b).astype(np.float32)
            mask_f[eo:eo + n_lo + n_hi][real] = mask[rids]
            gl = np.zeros(n_lo, np.int64)
            gl[:len(ids_lo)] = src[ids_lo]
            gh = np.zeros(n_hi, np.int64)
            gh[:len(ids_hi)] = src[ids_hi] - LOS
            idxl[ol:ol + n_lo] = gl
            idxh[oh:oh + n_hi] = gh
            ol += n_lo
            oh += n_hi

        inv_sb = np.ones((128, NBLK), np.float32)
        for b in range(NBLK):
            na = min(128, ASH - 128 * b)
            inv_sb[0:na, b] = invcnt[a0 + 128 * b: a0 + 128 * b + na]
        afT = np.zeros((AFD + 1, ASH), np.float32)
        afT[:AFD] = af[a0:a0 + ASH].T
        afT[AFD] = 1.0
        cbl = np.full((128, NBLK), -1.0, np.float32)  # crystal id per atom row
        for b in range(NBLK):
            na = min(128, ASH - 128 * b)
            cbl[0:na, b] = cb[a0 + 128 * b: a0 + 128 * b + na]

        m = {
            "eattrT": eattrT.astype(BF),
            "arow": arow_f.reshape(EP // 128, 128).T.copy(),
            "maskv": mask_f.reshape(EP // 128, 128).T.copy(),
            "idxlo": _wrap_idx(idxl), "invcnt": inv_sb,
            "afT": afT.astype(BF), "cbloc": cbl,
        }
        if IWH:
            m["idxhi"] = _wrap_idx(idxh)
        m.update(shared)
        in_maps.append(m)
    return meta, in_maps


def _build(meta, act=AFT.Silu, noop=False, no_gather=False, no_coll=False):
    cfg = meta["cfg"]
    N, M, AFD, EFD, NCRYS, L = (cfg[k] for k in ("N", "M", "AFD", "EFD", "NCRYS", "L"))
    ASH, NBLK, LOS, NG = meta["ASH"], meta["NBLK"], meta["LOS"], meta["NG"]
    EP, EPC, blocks = meta["EP"], meta["EPC"], meta["blocks"]
    IWL, IWH, ETILES = meta["IWL"], meta["IWH"], meta["ETILES"]

    nc = bacc.Bacc("TRN2", target_bir_lowering=False, debug=False, num_devices=NCORES,
                   num_swdge_queues=4)

    def din(name, shape, dt):
        return nc.dram_tensor(name, shape, dt, kind="ExternalInput")

    eattrT = din("eattrT", [EFD + 1, EP], BF16)
    arowd = din("arow", [128, ETILES], F32)
    maskd = din("maskv", [128, ETILES], F32)
    idxlo = din("idxlo", [16, IWL], I16)
    idxhi = din("idxhi", [16, IWH], I16) if IWH else None
    invcnt = din("invcnt", [128, NBLK], F32)
    afT = din("afT", [AFD + 1, ASH], BF16)
    cblocd = din("cbloc", [128, NBLK], F32)
    wts = {}
    for nm, sh, dt in [
        ("atomW", [AFD + 1, ND], BF16), ("edgeW", [EFD + 1, ED], BF16),
        ("we_dup", [128, L * HID], BF16), ("nw1b_dup", [128, L * ND], BF16),
        ("ws_all", [ND, L * HID], BF16), ("wd_all", [ND, L * HID], BF16),
        ("ew2_all", [HID, L * ED], BF16), ("nw1a_all", [ND, L * HID], BF16),
        ("nw2_all", [HID, L * ND], BF16), ("readW", [ND, RD], BF16),
        ("outW", [RD, 1], BF16), ("eb1", [128, L], F32), ("eb2p", [128, L], F32),
        ("nb1", [128, L], F32), ("nb2", [128, L], F32), ("readb", [RD, 1], F32),
        ("i64d", [128, 64], BF16), ("i128b", [128, 128], BF16),
        ("i128f", [128, 128], F32), ("invccnt", [GC, NG], F32),
    ]:
        wts[nm] = din(nm, sh, dt)
    y = nc.dram_tensor("y", [1, NCRYS], F32, kind="ExternalOutput")

    if noop:
        with tile.TileContext(nc) as tc:
            with tc.tile_pool(name="sbz", bufs=1) as sbz:
                yz = sbz.tile([1, NCRYS], F32, tag="yz")
                nc.gpsimd.memset(yz[:], 0.0)
                nc.sync.dma_start(y[:], yz[:])
        nc.compile()
        return nc

    with tile.TileContext(nc) as tc:
        with (
            tc.tile_pool(name="persist", bufs=1) as pp,
            tc.tile_pool(name="dram", bufs=1, space="DRAM") as dp,
        ):
            nc.gpsimd.load_library(library_config.mlp)
            w = {nm: pp.tile(t.shape, t.dtype, tag=nm, name=f"w_{nm}") for nm, t in wts.items()}
            for nm, t in wts.items():
                nc.sync.dma_start(w[nm][:], t[:])
            invcnt_sb = pp.tile([128, NBLK], F32, tag="invcnt_sb")
            nc.sync.dma_start(invcnt_sb[:], invcnt[:])
            stateT = pp.tile([128, EPC], BF16, tag="stateT")
            nodeT = pp.tile([128, ASH], F32, tag="nodeT")
            nodeTb = pp.tile([128, ASH], BF16, tag="nodeTb")
            adst = pp.tile([128, NBLK * 128], BF16, tag="adst")
            aggT = pp.tile([128, _cdiv(NBLK, 2) * 128], BF16, tag="aggT")
            idxsb = pp.tile([128, IWL], I16, tag="idxsb")
            for r in range(8):
                nc.sync.dma_start(idxsb[16 * r:16 * r + 16, :], idxlo[:])
            if IWH:
                idxsbh = pp.tile([128, IWH], I16, tag="idxsbh")
                for r in range(8):
                    nc.sync.dma_start(idxsbh[16 * r:16 * r + 16, :], idxhi[:])
            ssd = dp.tile([128, 2 * EP], BF16, name="ssd", tag="ssd")
            asrc_in = dp.tile([ASH, ND], BF16)
            asrc_fulls = [dp.tile([N, ND], BF16, addr_space="Shared", name=f"asrc_full{i}", tag=f"asrc_full{i}")
                          for i in range(L)]
            pool_in = dp.tile([NCRYS, ND], F32)
            pool_out = dp.tile([NCRYS, ND], F32, addr_space="Shared")

            def node_tables(lw, sbp, psp):
                """A_src shard -> bounce -> AllGather; A_dst blocks (layer lw)."""
                for t in range(NBLK):
                    na = min(128, ASH - 128 * t)
                    lhs = nodeTb[:, 128 * t:128 * t + na]
                    ps_s = psp.tile([128, 128], F32, tag="ps_s")
                    nc.tensor.matmul(ps_s[0:na, :], lhs, w["ws_all"][:, lw * HID:(lw + 1) * HID],
                                     start=True, stop=True)
                    asb = sbp.tile([128, 128], BF16, tag="asb")
                    nc.vector.tensor_copy(asb[0:na, :], ps_s[0:na, :])
                    nc.sync.dma_start(asrc_in[128 * t:128 * t + na, :], asb[0:na, :])
                    ps_d = psp.tile([128, 128], F32, tag="ps_d")
                    nc.tensor.matmul(ps_d[0:na, :], lhs, w["wd_all"][:, lw * HID:(lw + 1) * HID],
                                     start=True, stop=True)
                    nc.vector.tensor_copy(adst[0:na, 128 * t:128 * t + 128][:, 0:128],
                                          ps_d[0:na, :])
                if not no_coll:
                    nc.gpsimd.collective_compute(
                        "AllGather", mybir.AluOpType.bypass,
                        replica_groups=[list(range(NCORES))],
                        ins=[asrc_in[:].opt()], outs=[asrc_fulls[lw][:].opt()],
                    )

            # ---- init: projections + layer-0 tables ----
            with tc.tile_pool(name="sbi", bufs=3) as sbp, \
                 tc.tile_pool(name="psi", bufs=2, space="PSUM") as psp:
                for t in range(NBLK):
                    na = min(128, ASH - 128 * t)
                    aft = sbp.tile([AFD + 1, 128], BF16, tag="aft")
                    nc.sync.dma_start(aft[:, 0:na], afT[:, 128 * t:128 * t + na])
                    ps_n = psp.tile([128, 128], F32, tag="ps_n")
                    nc.tensor.matmul(ps_n[:, 0:na], w["atomW"][:], aft[:, 0:na],
                                     start=True, stop=True)
                    nc.vector.tensor_copy(nodeT[:, 128 * t:128 * t + na], ps_n[:, 0:na])
                    nc.vector.tensor_copy(nodeTb[:, 128 * t:128 * t + na], ps_n[:, 0:na])
                node_tables(0, sbp, psp)
            with tc.tile_pool(name="sbi2", bufs=3) as sbp, \
                 tc.tile_pool(name="psi2", bufs=2, space="PSUM") as psp:
                arow_sb = sbp.tile([128, ETILES], F32, tag="arow_sb", bufs=1)
                nc.sync.dma_start(arow_sb[:], arowd[:])
                mask_sb = sbp.tile([128, ETILES], F32, tag="mask_sb", bufs=1)
                nc.sync.dma_start(mask_sb[:], maskd[:])
                ramp128 = sbp.tile([128, 128], F32, tag="ramp128", bufs=1)
                nc.gpsimd.iota(ramp128[:], pattern=[[1, 128]], base=0,
                               channel_multiplier=0,
                               allow_small_or_imprecise_dtypes=True)
                for blk in blocks:
                    hr = slice(64, 128) if blk["half"] else slice(0, 64)
                    for (sco, eco, bco, n) in blk["chunks"]:
                        eat = sbp.tile([EFD + 1, 512], BF16, tag="eat")
                        nc.sync.dma_start(eat[:, 0:n], eattrT[:, eco:eco + n])
                        ps_e = psp.tile([128, 512], F32, tag="ps_e")
                        nc.tensor.matmul(ps_e[hr, 0:n], w["edgeW"][:], eat[:, 0:n],
                                         start=True, stop=True)
                        nc.vector.tensor_copy(stateT[hr, sco:sco + n], ps_e[hr, 0:n])
                        # build scatter one-hots for this chunk into DRAM ssd:
                        # S^T [atom, edge] (unmasked) and S tiles [edge, atom] (masked)
                        sb_s = sbp.tile([128, 512], BF16, tag="sb_s")
                        ps_tr = psp.tile([128, 512], BF16, tag="ps_tr")
                        sb_st = sbp.tile([128, 512], BF16, tag="sb_st")
                        for j in range(n // 128):
                            kk = eco // 128 + j
                            oh = sbp.tile([128, 128], BF16, tag="oh")
                            nc.vector.tensor_scalar(
                                oh[:], ramp128[:], arow_sb[:, kk:kk + 1], None,
                                mybir.AluOpType.is_equal)
                            nc.vector.tensor_scalar(
                                sb_s[:, 128 * j:128 * j + 128], ramp128[:],
                                arow_sb[:, kk:kk + 1], mask_sb[:, kk:kk + 1],
                                mybir.AluOpType.is_equal, mybir.AluOpType.mult)
                            nc.tensor.transpose(ps_tr[:, 128 * j:128 * j + 128],
                                                oh[:], w["i128b"][:])
                        nc.vector.tensor_copy(sb_st[:, 0:n], ps_tr[:, 0:n])
                        nc.sync.dma_start(ssd[:, 2 * eco:2 * eco + n], sb_st[:, 0:n])
                        nc.sync.dma_start(ssd[:, 2 * eco + n:2 * eco + 2 * n],
                                          sb_s[:, 0:n])

            # ---- layers ----
            for l in range(L):
                with tc.tile_pool(name=f"sbe{l}", bufs=3) as sbp, \
                     tc.tile_pool(name=f"pse{l}", bufs=2, space="PSUM") as psp, \
                     tc.tile_pool(name=f"psg{l}", bufs=2, space="PSUM") as psg:
                    for blk in blocks:
                        b = blk["b"]
                        hr = slice(64, 128) if blk["half"] else slice(0, 64)
                        ba = min(128, ASH - 128 * b)
                        asrc_full = asrc_fulls[l]
                        gt = sbp.tile([128, 1, blk["nblk_e"]], BF16, tag="gt", bufs=2)
                        if blk["n_lo"] and not no_gather:
                            io = sum(bb["n_lo"] for bb in blocks[:b]) // 16
                            nc.gpsimd.dma_gather(
                                gt[:, :, 0:blk["n_lo"]], asrc_full[0:LOS, :],
                                idxsb[:, io:io + blk["n_lo"] // 16],
                                blk["n_lo"], blk["n_lo"], ND, transpose=True,
                                queue_num=(2 * b) % 4)
                        if blk["n_hi"] and not no_gather:
                            io = sum(bb["n_hi"] for bb in blocks[:b]) // 16
                            nc.gpsimd.dma_gather(
                                gt[:, :, blk["n_lo"]:], asrc_full[LOS:N, :],
                                idxsbh[:, io:io + blk["n_hi"] // 16],
                                blk["n_hi"], blk["n_hi"], ND, transpose=True,
                                queue_num=(2 * b + 1) % 4)
                        ps_agg = psg.tile([128, 64], F32, tag="agg")
                        nchunk = len(blk["chunks"])
                        e0 = blk["chunks"][0][1]
                        ssb = sbp.tile([128, 2 * blk["nblk_e"]], BF16, tag="ssb", bufs=2)
                        nc.sync.dma_start(ssb[:, 0:2 * blk["nblk_e"]],
                                          ssd[:, 2 * e0:2 * e0 + 2 * blk["nblk_e"]])
                        for ci, (sco, eco, bco, n) in enumerate(blk["chunks"]):
                            sst = ssb[:, 2 * (eco - e0):2 * (eco - e0) + 2 * n]
                            ps_h = psp.tile([128, 512], F32, tag="ph")
                            nc.tensor.matmul(ps_h[:, 0:n], adst[0:ba, 128 * b:128 * b + 128],
                                             sst[0:ba, 0:n], start=True, stop=False)  # S^T chunk
                            nc.tensor.matmul(ps_h[:, 0:n], w["we_dup"][hr, l * HID:(l + 1) * HID],
                                             stateT[hr, sco:sco + n], start=False,
                                             stop=no_gather)
                            if not no_gather:
                                nc.tensor.matmul(ps_h[:, 0:n], w["i128b"][:],
                                                 gt[:, 0, bco:bco + n], start=False, stop=True)
                            ht = sbp.tile([128, 512], BF16, tag="ht")
                            nc.scalar.activation(ht[:, 0:n], ps_h[:, 0:n], act,
                                                 bias=w["eb1"][:, l:l + 1])
                            ps_dd = psp.tile([128, 512], F32, tag="pd")
                            nc.tensor.matmul(ps_dd[hr, 0:n], w["i64d"][hr, :],
                                             stateT[hr, sco:sco + n], start=True, stop=False)
                            nc.tensor.matmul(ps_dd[hr, 0:n], w["ew2_all"][:, l * ED:(l + 1) * ED],
                                             ht[:, 0:n], start=False, stop=True)
                            nc.vector.tensor_scalar(stateT[hr, sco:sco + n],
                                                    ps_dd[hr, 0:n],
                                                    w["eb2p"][hr, l:l + 1], None,
                                                    mybir.AluOpType.add)
                            ps_t = psp.tile([128, 256], BF16, tag="pt", bufs=1)
                            for j in range(n // 128):
                                nc.tensor.transpose(
                                    ps_t[:, 64 * j:64 * j + 64],
                                    stateT[hr, sco + 128 * j:sco + 128 * j + 128],
                                    w["i64d"][hr, :])
                            nn = sbp.tile([128, 256], BF16, tag="nn")
                            nc.vector.tensor_copy(nn[:, 0:64 * (n // 128)], ps_t[:, 0:64 * (n // 128)])
                            for j in range(n // 128):
                                nc.tensor.matmul(
                                    ps_agg[:],
                                    sst[:, n + 128 * j:n + 128 * j + 128],
                                    nn[:, 64 * j:64 * j + 64],
                                    start=(ci == 0 and j == 0),
                                    stop=(ci == nchunk - 1 and j == n // 128 - 1))
                        agnb = sbp.tile([128, 64], BF16, tag="agnb")
                        nc.scalar.activation(agnb[:], ps_agg[:], AFT.Identity,
                                             scale=invcnt_sb[:, b:b + 1])
                        ps_at = psp.tile([128, 128], BF16, tag="pat", bufs=1)
                        hr2 = slice(64, 128) if b % 2 else slice(0, 64)
                        nc.tensor.transpose(ps_at[hr2, :], agnb[:], w["i128b"][:])
                        nc.vector.tensor_copy(aggT[hr2, (b // 2) * 128:(b // 2) * 128 + 128],
                                              ps_at[hr2, :])
                # node MLP + next-layer tables
                with tc.tile_pool(name=f"sbn{l}", bufs=3) as sbp, \
                     tc.tile_pool(name=f"psn{l}", bufs=2, space="PSUM") as psp:
                    for t in range(NBLK):
                        na = min(128, ASH - 128 * t)
                        hr2 = slice(64, 128) if t % 2 else slice(0, 64)
                        ps_hn = psp.tile([128, 128], F32, tag="hn")
                        nc.tensor.matmul(ps_hn[:, 0:na],
                                         w["nw1a_all"][:, l * HID:(l + 1) * HID],
                                         nodeTb[:, 128 * t:128 * t + na],
                                         start=True, stop=False)
                        nc.tensor.matmul(ps_hn[:, 0:na],
                                         w["nw1b_dup"][hr2, l * HID:(l + 1) * HID],
                                         aggT[hr2, (t // 2) * 128:(t // 2) * 128 + na],
                                         start=False, stop=True)
                        hn = sbp.tile([128, 128], BF16, tag="hn_s")
                        nc.scalar.activation(hn[:, 0:na], ps_hn[:, 0:na], act,
                                             bias=w["nb1"][:, l:l + 1])
                        ps_nd = psp.tile([128, 128], F32, tag="ndl")
                        nc.tensor.matmul(ps_nd[:, 0:na],
                                         w["nw2_all"][:, l * ND:(l + 1) * ND],
                                         hn[:, 0:na], start=True, stop=False)
                        nc.tensor.matmul(ps_nd[:, 0:na], w["i128f"][:],
                                         nodeT[:, 128 * t:128 * t + na], start=False, stop=True)
                        nc.scalar.activation(nodeT[:, 128 * t:128 * t + na], ps_nd[:, 0:na],
                                             AFT.Identity, bias=w["nb2"][:, l:l + 1])
                        nc.vector.tensor_copy(nodeTb[:, 128 * t:128 * t + na],
                                              nodeT[:, 128 * t:128 * t + na])
                    if l < L - 1:
                        node_tables(l + 1, sbp, psp)

            # ---- pooling ----
            with tc.tile_pool(name="sbt", bufs=3) as sbt, \
                 tc.tile_pool(name="pst", bufs=2, space="PSUM") as pst:
                nnat_all = pp.tile([128, NBLK * 128], BF16, tag="nnat_all")
                for t in range(NBLK):
                    na = min(128, ASH - 128 * t)
                    ps_tr = pst.tile([128, 128], F32, tag="ptr")
                    nc.tensor.transpose(ps_tr[0:na, :], nodeT[:, 128 * t:128 * t + na],
                                        w["i128f"][:])
                    nc.vector.tensor_copy(nnat_all[0:na, 128 * t:128 * t + 128][:, 0:128],
                                          ps_tr[0:na, :])
            with tc.tile_pool(name="sbp", bufs=3) as sbp, \
                 tc.tile_pool(name="psp", bufs=1, space="PSUM") as psp:
                pools = [psp.tile([128, 128], F32, tag=f"pool{g}", name=f"pool{g}") for g in range(NG)]
                rampP = sbp.tile([128, NG * GC], F32, tag="rampP", bufs=1)
                nc.gpsimd.iota(rampP[:], pattern=[[1, NG * GC]], base=0,
                               channel_multiplier=0,
                               allow_small_or_imprecise_dtypes=True)
                cbloc_sb = sbp.tile([128, NBLK], F32, tag="cbloc_sb", bufs=1)
                nc.sync.dma_start(cbloc_sb[:], cblocd[:])
                for t in range(NBLK):
                    na = min(128, ASH - 128 * t)
                    pmt = sbp.tile([128, NG * GC], BF16, tag="pmt")
                    nc.vector.tensor_scalar(pmt[:], rampP[:], cbloc_sb[:, t:t + 1],
                                            None, mybir.AluOpType.is_equal)
                    for g in range(NG):
                        gc = min(GC, NCRYS - g * GC)
                        nc.tensor.matmul(pools[g][0:gc, :], pmt[0:na, g * GC:g * GC + gc],
                                         nnat_all[0:na, 128 * t:128 * t + 128][:, 0:128],
                                         start=(t == 0), stop=(t == NBLK - 1))
                for g in range(NG):
                    gc = min(GC, NCRYS - g * GC)
                    pev = sbp.tile([128, 128], F32, tag="pev")
                    nc.vector.tensor_copy(pev[0:gc, :], pools[g][0:gc, :])
                    nc.sync.dma_start(pool_in[g * GC:g * GC + gc, :], pev[0:gc, :])
                nc.gpsimd.collective_compute(
                    "AllReduce", mybir.AluOpType.add,
                    replica_groups=[list(range(NCORES))],
                    ins=[pool_in[:].opt()], outs=[pool_out[:].opt()],
                )

            # ---- readout (replicated) ----
            with tc.tile_pool(name="sbr", bufs=2) as sbp, \
                 tc.tile_pool(name="psr", bufs=2, space="PSUM") as psp:
                for g in range(NG):
                    gc = min(GC, NCRYS - g * GC)
                    pg = sbp.tile([128, 128], F32, tag="pg")
                    nc.sync.dma_start(pg[0:gc, :], pool_out[g * GC:g * GC + gc, :])
                    mean = sbp.tile([128, 128], BF16, tag="mean")
                    nc.scalar.activation(mean[0:gc, :], pg[0:gc, :], AFT.Identity,
                                         scale=w["invccnt"][0:gc, g:g + 1])
                    ps_mt = psp.tile([128, 128], BF16, tag="pmt2")
                    nc.tensor.transpose(ps_mt[:, 0:gc], mean[0:gc, :], w["i128b"][0:gc, 0:gc])
                    meanT = sbp.tile([128, 128], BF16, tag="meanT")
                    nc.vector.tensor_copy(meanT[:, 0:gc], ps_mt[:, 0:gc])
                    ps_hr = psp.tile([128, 128], F32, tag="phr")
                    nc.tensor.matmul(ps_hr[:, 0:gc], w["readW"][:], meanT[:, 0:gc],
                                     start=True, stop=True)
                    hrT = sbp.tile([128, 128], BF16, tag="hrT")
                    nc.scalar.activation(hrT[:, 0:gc], ps_hr[:, 0:gc], act,
                                         bias=w["readb"][:])
                    ps_y = psp.tile([128, 128], F32, tag="py")
                    nc.tensor.matmul(ps_y[0:1, 0:gc], w["outW"][:], hrT[:, 0:gc],
                                     start=True, stop=True)
                    ysb = sbp.tile([1, 128], F32, tag="ysb")
                    nc.scalar.activation(ysb[0:1, 0:gc], ps_y[0:1, 0:gc], AFT.Copy,
                                         bias=meta["out_b"])
                    nc.sync.dma_start(y[0:1, g * GC:g * GC + gc], ysb[0:1, 0:gc])

    nc.compile()
    return nc


def run_cores(meta, in_maps, act=AFT.Silu, sim=False):
    nc = _build(meta, act=act)
    if sim:
        from concourse.bass_interp import MultiCoreSim
        s = MultiCoreSim(nc, NCORES, trace=False)
        for k in range(NCORES):
            for nm, arr in in_maps[k].items():
                s.cores[k].tensor(nm)[:] = arr
        s.simulate(check_with_hw=False)
        return [{"y": np.array(s.cores[k].tensor("y"))} for k in range(NCORES)], None
    from concourse import bass_utils
    res = bass_utils.run_bass_kernel_spmd(nc, in_maps, core_ids=list(range(NCORES)))
    return res.results, res


def kernel(**inputs):
    cfg = dict(FULL_CFG)
    n, m = np.asarray(inputs["nbr_fea_idx"]).shape
    cfg["N"], cfg["M"] = int(n), int(m)
    cfg["AFD"] = int(np.asarray(inputs["atom_fea"]).shape[1])
    cfg["EFD"] = int(np.asarray(inputs["nbr_fea"]).shape[2])
    cfg["NCRYS"] = int(inputs["num_crystals"])
    cfg["L"] = int(np.asarray(inputs["eW1"]).shape[0])
    meta, in_maps = _prep(inputs, cfg)
    results, _ = run_cores(meta, in_maps)
    return np.asarray(results[0]["y"], np.float32).reshape(cfg["NCRYS"], 1)



# revision 38
# speedup vs baseline: 4.9267x; 1.4022x over previous
"""CrystalGraphALIGNN Trainium2 kernel (8 NeuronCores, SPMD).

Strategy: dst-shard edges across cores (atom v owned by core v // (N/8); edge
(i,j) owned by the core of its dst). Per core, edges are sorted by dst and
grouped into 128-atom blocks so that:
  - the dst-side expansion A_dst[dst(e)] is a block-local one-hot matmul
    (S^T streamed from DRAM),
  - the scatter-mean aggregation is a one-hot matmul into PSUM (S streamed),
  - only the src side needs a true random gather: per-edge rows of
    A_src = node @ W_src, fetched with dma_gather (transposed, bf16) from a
    DRAM table that is refreshed once per layer via AllGather.
Node states and the node MLP stay fully shard-local; crystal pooling is a
one-hot matmul + a single AllReduce, readout replicated on every core.
"""

import numpy as np
import ml_dtypes

import concourse.bass as bass
import concourse.bacc as bacc
import concourse.mybir as mybir
import concourse.tile as tile
from concourse import library_config

F32 = mybir.dt.float32
BF16 = mybir.dt.bfloat16
I16 = mybir.dt.int16
AFT = mybir.ActivationFunctionType
BF = ml_dtypes.bfloat16

NCORES = 8
ED, ND, HID, RD = 64, 128, 128, 128
EDGE_THRESH = 1e-6
GC = 125  # crystals per pooling group

FULL_CFG = dict(N=50000, M=12, AFD=92, EFD=41, NCRYS=1000, L=4)


def _cdiv(a, b):
    return (a + b - 1) // b


def _wrap_idx(flat):
    """int16 flat idx -> [16, len/16] wrapped layout (replicated to 128 on device)."""
    n = len(flat)
    assert n % 16 == 0
    return flat.reshape(n // 16, 16).T.astype(np.int16)


def _prep(inputs, cfg):
    N, M, AFD, EFD, NCRYS, L = (cfg[k] for k in ("N", "M", "AFD", "EFD", "NCRYS", "L"))
    ASH = N // NCORES
    NBLK = _cdiv(ASH, 128)
    LOS = min(25000, N)  # src index split for int16 gather indices
    NG = _cdiv(NCRYS, GC)

    af = np.asarray(inputs["atom_fea"], np.float32)
    nf = np.asarray(inputs["nbr_fea"], np.float32)
    nidx = np.asarray(inputs["nbr_fea_idx"]).astype(np.int64)
    cb = np.asarray(inputs["crystal_batch"]).astype(np.int64)

    E = N * M
    dst = np.clip(nidx.reshape(-1), 0, N - 1)
    src = np.repeat(np.arange(N, dtype=np.int64), M)
    ea = nf.reshape(E, EFD)
    mask = (np.abs(ea).sum(1) > EDGE_THRESH).astype(np.float32)

    cnt = np.bincount(dst, weights=mask, minlength=N)
    invcnt = (1.0 / np.maximum(cnt, 1.0)).astype(np.float32)
    ccnt = np.bincount(cb, minlength=NCRYS).astype(np.float32)
    invccnt = (1.0 / np.maximum(ccnt, 1.0)).astype(np.float32)

    core_of = dst // ASH
    dloc = dst - core_of * ASH
    blk_of = dloc // 128

    # per-core, per-block, lo/hi edge id lists
    lists = [[[None, None] for _ in range(NBLK)] for _ in range(NCORES)]
    order = np.lexsort((dst, blk_of + core_of * NBLK))  # group by (core, blk)
    for k in range(NCORES):
        esel = order[(core_of[order] == k)]
        for b in range(NBLK):
            eb = esel[blk_of[esel] == b]
            lists[k][b][0] = eb[src[eb] < LOS]
            lists[k][b][1] = eb[src[eb] >= LOS]

    T_lo = np.zeros(NBLK, np.int64)
    T_hi = np.zeros(NBLK, np.int64)
    for b in range(NBLK):
        T_lo[b] = max(_cdiv(max(len(lists[k][b][0]) for k in range(NCORES)), 128), 1)
        T_hi[b] = _cdiv(max(len(lists[k][b][1]) for k in range(NCORES)), 128)
        if (T_lo[b] + T_hi[b]) % 2:
            if N > LOS:
                T_hi[b] += 1
            else:
                T_lo[b] += 1

    # geometry: edge-col space (block-major), state-col space (per half),
    # chunk list entries: (b, half, state_col, edge_col, blk_edge_col, n)
    BHALF = NBLK // 2
    ecol = np.zeros(NBLK + 1, np.int64)
    for b in range(NBLK):
        ecol[b + 1] = ecol[b] + (T_lo[b] + T_hi[b]) * 128
    EP = int(ecol[NBLK])
    scol = np.zeros(NBLK, np.int64)
    acc = [0, 0]
    blocks = []
    for b in range(NBLK):
        half = 0 if b < BHALF else 1
        scol[b] = acc[half]
        nb_e = (T_lo[b] + T_hi[b]) * 128
        acc[half] += nb_e
        tiles = (T_lo[b] + T_hi[b])
        chunks = []
        off = 0
        while tiles > 0:
            t = 4 if tiles >= 4 else tiles
            chunks.append((int(scol[b] + off), int(ecol[b] + off), off, t * 128))
            off += t * 128
            tiles -= t
        blocks.append(dict(b=b, half=half, nblk_e=nb_e, chunks=chunks,
                           n_lo=int(T_lo[b] * 128), n_hi=int(T_hi[b] * 128)))
    EPC = max(acc)
    IWL = sum(int(t) * 8 for t in T_lo)
    IWH = sum(int(t) * 8 for t in T_hi)

    meta = dict(cfg=cfg, ASH=ASH, NBLK=NBLK, LOS=LOS, NG=NG, EP=EP, EPC=EPC,
                BHALF=BHALF, blocks=blocks, IWL=IWL, IWH=IWH, ETILES=EP // 128,
                out_b=float(np.asarray(inputs["out_b"]).reshape(-1)[0]))

    # shared weights
    eW1 = np.asarray(inputs["eW1"], np.float32)
    eW2 = np.asarray(inputs["eW2"], np.float32)
    nW1 = np.asarray(inputs["nW1"], np.float32)
    nW2 = np.asarray(inputs["nW2"], np.float32)

    def bfc(x):
        return np.ascontiguousarray(x, np.float32).astype(BF)

    # int8 feature shipping: fold the dequant scale into the projection
    # weights; bias row ships as q=127 with weight row bias/127.
    s_af = float(np.abs(af).max()) / 127.0
    s_ea = float(np.abs(ea).max()) / 127.0
    atomW93 = np.zeros((AFD + 1, ND), np.float32)
    atomW93[:AFD] = np.asarray(inputs["atom_W"], np.float32) * s_af
    atomW93[AFD] = np.asarray(inputs["atom_b"], np.float32) / 127.0
    edgeW42 = np.zeros((EFD + 1, ED), np.float32)
    edgeW42[:EFD] = np.asarray(inputs["edge_W"], np.float32) * s_ea
    edgeW42[EFD] = np.asarray(inputs["edge_b"], np.float32) / 127.0

    we_dup = np.zeros((128, L * HID), np.float32)
    nw1b_dup = np.zeros((128, L * ND), np.float32)
    for l in range(L):
        we_dup[0:64, l * HID:(l + 1) * HID] = eW1[l, 0:ED]
        we_dup[64:128, l * HID:(l + 1) * HID] = eW1[l, 0:ED]
        nw1b_dup[0:64, l * ND:(l + 1) * ND] = nW1[l, ND:ND + ED]
        nw1b_dup[64:128, l * ND:(l + 1) * ND] = nW1[l, ND:ND + ED]
    # per-layer [ws | wd] side by side so node_tables needs one matmul
    wsd_all = np.concatenate(
        [np.concatenate([eW1[l, ED:ED + ND], eW1[l, ED + ND:]], 1) for l in range(L)], 1)
    ew2_all = np.concatenate([eW2[l] for l in range(L)], 1)                 # [128, L*64]
    nw1a_all = np.concatenate([nW1[l, 0:ND] for l in range(L)], 1)          # [128, L*128]
    nw2_all = np.concatenate([nW2[l] for l in range(L)], 1)                 # [128, L*128]

    eb1 = np.asarray(inputs["eb1"], np.float32).T.copy()                    # [128, L]
    eb2p = np.zeros((128, L), np.float32)
    eb2p[0:64] = np.asarray(inputs["eb2"], np.float32).T
    eb2p[64:128] = eb2p[0:64]
    nb1 = np.asarray(inputs["nb1"], np.float32).T.copy()
    nb2 = np.asarray(inputs["nb2"], np.float32).T.copy()

    shared = {
        "atomW": bfc(atomW93), "edgeW": bfc(edgeW42),
        "we_dup": bfc(we_dup), "nw1b_dup": bfc(nw1b_dup),
        "wsd_all": bfc(wsd_all), "ew2_all": bfc(ew2_all),
        "nw1a_all": bfc(nw1a_all), "nw2_all": bfc(nw2_all),
        "readW": bfc(np.asarray(inputs["read_W"])), "outW": bfc(np.asarray(inputs["out_W"])),
        "eb1": eb1, "eb2p": eb2p, "nb1": nb1, "nb2": nb2,
        "readb": np.asarray(inputs["read_b"], np.float32).reshape(RD, 1),
        "invccnt": np.pad(invccnt, (0, NG * GC - NCRYS)).reshape(NG, GC).T.copy(),  # [125, NG]
    }

    in_maps = []
    for k in range(NCORES):
        a0 = k * ASH
        eattrT = np.zeros((EFD + 1, EP), np.float32)
        arow_f = np.full(EP, -1.0, np.float32)   # block-local dst row per edge col
        mask_f = np.zeros(EP, np.float32)
        idxl = np.zeros(sum(int(t) * 128 for t in T_lo), np.int64)
        idxh = np.zeros(sum(int(t) * 128 for t in T_hi), np.int64)
        ol = oh = 0
        for blk in blocks:
            b = blk["b"]
            ids_lo, ids_hi = lists[k][b]
            n_lo, n_hi = blk["n_lo"], blk["n_hi"]
            eo = int(ecol[b])
            ids = np.full(n_lo + n_hi, -1, np.int64)
            ids[:len(ids_lo)] = ids_lo
            ids[n_lo:n_lo + len(ids_hi)] = ids_hi
            real = ids >= 0
            rids = ids[real]
            eattrT[:EFD, eo:eo + n_lo + n_hi][:, real] = np.round(ea[rids].T / s_ea)
            eattrT[EFD, eo:eo + n_lo + n_hi][real] = 127.0
            # fold the scatter-mean 1/cnt into the S-tile values: the one-hot
            # column is exactly dst(e), so S[e,a] = mask*invcnt[dst] makes the
            # aggregation matmul produce the mean directly.
            arow_f[eo:eo + n_lo + n_hi][real] = (dloc[rids] - 128 * b).astype(np.float32)
            mask_f[eo:eo + n_lo + n_hi][real] = mask[rids] * invcnt[dst[rids]]
            gl = np.zeros(n_lo, np.int64)
            gl[:len(ids_lo)] = src[ids_lo]
            gh = np.zeros(n_hi, np.int64)
            gh[:len(ids_hi)] = src[ids_hi] - LOS
            idxl[ol:ol + n_lo] = gl
            idxh[oh:oh + n_hi] = gh
            ol += n_lo
            oh += n_hi

        afT = np.zeros((AFD + 1, ASH), np.float32)
        afT[:AFD] = np.round(af[a0:a0 + ASH].T / s_af)
        afT[AFD] = 127.0
        cbl = np.full((128, NBLK), -1.0, np.float32)  # crystal id per atom row
        for b in range(NBLK):
            na = min(128, ASH - 128 * b)
            cbl[0:na, b] = cb[a0 + 128 * b: a0 + 128 * b + na]

        m = {
            "eattrT": eattrT.astype(np.int8),
            "arow": arow_f.reshape(EP // 128, 128).T.astype(BF),
            "maskv": mask_f.reshape(EP // 128, 128).T.astype(BF),
            "idxlo": _wrap_idx(idxl),
            "afT": afT.astype(np.int8), "cbloc": cbl,
        }
        if IWH:
            m["idxhi"] = _wrap_idx(idxh)
        m.update(shared)
        in_maps.append(m)
    return meta, in_maps


def _build(meta, act=AFT.Silu, noop=False, no_gather=False, no_coll=False):
    cfg = meta["cfg"]
    N, M, AFD, EFD, NCRYS, L = (cfg[k] for k in ("N", "M", "AFD", "EFD", "NCRYS", "L"))
    ASH, NBLK, LOS, NG = meta["ASH"], meta["NBLK"], meta["LOS"], meta["NG"]
    EP, EPC, blocks = meta["EP"], meta["EPC"], meta["blocks"]
    IWL, IWH, ETILES = meta["IWL"], meta["IWH"], meta["ETILES"]

    nc = bacc.Bacc("TRN2", target_bir_lowering=False, debug=False, num_devices=NCORES,
                   num_swdge_queues=4)

    def din(name, shape, dt):
        return nc.dram_tensor(name, shape, dt, kind="ExternalInput")

    I8 = mybir.dt.int8
    eattrT = din("eattrT", [EFD + 1, EP], I8)
    arowd = din("arow", [128, ETILES], BF16)
    maskd = din("maskv", [128, ETILES], BF16)
    idxlo = din("idxlo", [16, IWL], I16)
    idxhi = din("idxhi", [16, IWH], I16) if IWH else None
    afT = din("afT", [AFD + 1, ASH], I8)
    cblocd = din("cbloc", [128, NBLK], F32)
    wts = {}
    for nm, sh, dt in [
        ("atomW", [AFD + 1, ND], BF16), ("edgeW", [EFD + 1, ED], BF16),
        ("we_dup", [128, L * HID], BF16), ("nw1b_dup", [128, L * ND], BF16),
        ("wsd_all", [ND, L * 2 * HID], BF16),
        ("ew2_all", [HID, L * ED], BF16), ("nw1a_all", [ND, L * HID], BF16),
        ("nw2_all", [HID, L * ND], BF16), ("readW", [ND, RD], BF16),
        ("outW", [RD, 1], BF16), ("eb1", [128, L], F32), ("eb2p", [128, L], F32),
        ("nb1", [128, L], F32), ("nb2", [128, L], F32), ("readb", [RD, 1], F32),
        ("invccnt", [GC, NG], F32),
    ]:
        wts[nm] = din(nm, sh, dt)
    y = nc.dram_tensor("y", [1, NCRYS], F32, kind="ExternalOutput")

    if noop:
        with tile.TileContext(nc) as tc:
            with tc.tile_pool(name="sbz", bufs=1) as sbz:
                yz = sbz.tile([1, NCRYS], F32, tag="yz")
                nc.gpsimd.memset(yz[:], 0.0)
                nc.sync.dma_start(y[:], yz[:])
        nc.compile()
        return nc

    with tile.TileContext(nc) as tc:
        with (
            tc.tile_pool(name="persist", bufs=1) as pp,
            tc.tile_pool(name="dram", bufs=1, space="DRAM") as dp,
        ):
            nc.gpsimd.load_library(library_config.mlp)
            w = {nm: pp.tile(t.shape, t.dtype, tag=nm, name=f"w_{nm}") for nm, t in wts.items()}
            for nm, t in wts.items():
                nc.sync.dma_start(w[nm][:], t[:])
            # identities built on device (not shipped)
            ramp128 = pp.tile([128, 128], F32, tag="ramp128")
            nc.gpsimd.iota(ramp128[:], pattern=[[1, 128]], base=0,
                           channel_multiplier=0, allow_small_or_imprecise_dtypes=True)
            pidx = pp.tile([128, 1], F32, tag="pidx")
            nc.gpsimd.iota(pidx[:], pattern=[[1, 1]], base=0,
                           channel_multiplier=1, allow_small_or_imprecise_dtypes=True)
            w["i128f"] = pp.tile([128, 128], F32, tag="i128f", name="w_i128f")
            nc.vector.tensor_scalar(w["i128f"][:], ramp128[:], pidx[:], None,
                                    mybir.AluOpType.is_equal)
            w["i128b"] = pp.tile([128, 128], BF16, tag="i128b", name="w_i128b")
            nc.vector.tensor_copy(w["i128b"][:], w["i128f"][:])
            w["i64d"] = pp.tile([128, 64], BF16, tag="i64d", name="w_i64d")
            nc.vector.tensor_copy(w["i64d"][0:64, :], w["i128f"][0:64, 0:64])
            nc.vector.tensor_copy(w["i64d"][64:128, :], w["i128f"][64:128, 64:128])
            stateT = pp.tile([128, EPC], BF16, tag="stateT")
            nodeT = pp.tile([128, ASH], F32, tag="nodeT")
            nodeTb = pp.tile([128, ASH], BF16, tag="nodeTb")
            adst = pp.tile([128, NBLK * 128], BF16, tag="adst")
            aggT = pp.tile([128, _cdiv(NBLK, 2) * 128], BF16, tag="aggT")
            idxsb = pp.tile([128, IWL], I16, tag="idxsb")
            for r in range(8):
                nc.sync.dma_start(idxsb[16 * r:16 * r + 16, :], idxlo[:])
            if IWH:
                idxsbh = pp.tile([128, IWH], I16, tag="idxsbh")
                for r in range(8):
                    nc.sync.dma_start(idxsbh[16 * r:16 * r + 16, :], idxhi[:])
            ssd = dp.tile([128, 2 * EP], BF16, name="ssd", tag="ssd")
            asrc_in = dp.tile([ASH, ND], BF16)
            asrc_fulls = [dp.tile([N, ND], BF16, addr_space="Shared", name=f"asrc_full{i}", tag=f"asrc_full{i}")
                          for i in range(L)]
            pool_in = dp.tile([NCRYS, ND], F32)
            pool_out = dp.tile([NCRYS, ND], F32, addr_space="Shared")

            def node_tables(lw, sbp, psp):
                """A_src shard -> bounce -> AllGather; A_dst blocks (layer lw)."""
                for t in range(NBLK):
                    na = min(128, ASH - 128 * t)
                    lhs = nodeTb[:, 128 * t:128 * t + na]
                    ps_sd = psp.tile([128, 256], F32, tag="ps_sd")
                    nc.tensor.matmul(ps_sd[0:na, :], lhs,
                                     w["wsd_all"][:, lw * 256:(lw + 1) * 256],
                                     start=True, stop=True)
                    asb = sbp.tile([128, 128], BF16, tag="asb")
                    nc.vector.tensor_copy(asb[0:na, :], ps_sd[0:na, 0:128])
                    nc.sync.dma_start(asrc_in[128 * t:128 * t + na, :], asb[0:na, :])
                    nc.vector.tensor_copy(adst[0:na, 128 * t:128 * t + 128][:, 0:128],
                                          ps_sd[0:na, 128:256])
                if not no_coll:
                    nc.gpsimd.collective_compute(
                        "AllGather", mybir.AluOpType.bypass,
                        replica_groups=[list(range(NCORES))],
                        ins=[asrc_in[:].opt()], outs=[asrc_fulls[lw][:].opt()],
                    )

            # ---- init: projections + layer-0 tables ----
            with tc.tile_pool(name="sbi", bufs=3) as sbp, \
                 tc.tile_pool(name="psi", bufs=2, space="PSUM") as psp:
                for t in range(NBLK):
                    na = min(128, ASH - 128 * t)
                    aft8 = sbp.tile([AFD + 1, 128], I8, tag="aft8")
                    nc.sync.dma_start(aft8[:, 0:na], afT[:, 128 * t:128 * t + na])
                    aft = sbp.tile([AFD + 1, 128], BF16, tag="aft")
                    nc.vector.tensor_copy(aft[:, 0:na], aft8[:, 0:na])
                    ps_n = psp.tile([128, 128], F32, tag="ps_n")
                    nc.tensor.matmul(ps_n[:, 0:na], w["atomW"][:], aft[:, 0:na],
                                     start=True, stop=True)
                    nc.vector.tensor_copy(nodeT[:, 128 * t:128 * t + na], ps_n[:, 0:na])
                    nc.vector.tensor_copy(nodeTb[:, 128 * t:128 * t + na], ps_n[:, 0:na])
                node_tables(0, sbp, psp)
            with tc.tile_pool(name="sbi2", bufs=3) as sbp, \
                 tc.tile_pool(name="psi2", bufs=2, space="PSUM") as psp:
                arow_l = sbp.tile([128, ETILES], BF16, tag="arow_l", bufs=1)
                nc.sync.dma_start(arow_l[:], arowd[:])
                arow_sb = sbp.tile([128, ETILES], F32, tag="arow_sb", bufs=1)
                nc.vector.tensor_copy(arow_sb[:], arow_l[:])
                mask_l = sbp.tile([128, ETILES], BF16, tag="mask_l", bufs=1)
                nc.sync.dma_start(mask_l[:], maskd[:])
                mask_sb = sbp.tile([128, ETILES], F32, tag="mask_sb", bufs=1)
                nc.vector.tensor_copy(mask_sb[:], mask_l[:])
                MAXBE = max(bb["nblk_e"] for bb in blocks)
                for blk in blocks:
                    hr = slice(64, 128) if blk["half"] else slice(0, 64)
                    e0b = blk["chunks"][0][1]
                    eat8 = sbp.tile([EFD + 1, MAXBE], I8, tag="eat8", bufs=2)
                    nc.sync.dma_start(eat8[:, 0:blk["nblk_e"]],
                                      eattrT[:, e0b:e0b + blk["nblk_e"]])
                    for (sco, eco, bco, n) in blk["chunks"]:
                        eat = sbp.tile([EFD + 1, 512], BF16, tag="eat")
                        nc.vector.tensor_copy(eat[:, 0:n],
                                              eat8[:, eco - e0b:eco - e0b + n])
                        ps_e = psp.tile([128, 512], F32, tag="ps_e")
                        nc.tensor.matmul(ps_e[hr, 0:n], w["edgeW"][:], eat[:, 0:n],
                                         start=True, stop=True)
                        nc.vector.tensor_copy(stateT[hr, sco:sco + n], ps_e[hr, 0:n])
                        # build scatter one-hots for this chunk into DRAM ssd:
                        # S^T [atom, edge] (unmasked) and S tiles [edge, atom] (masked)
                        sb_s = sbp.tile([128, 512], BF16, tag="sb_s")
                        ps_tr = psp.tile([128, 512], BF16, tag="ps_tr")
                        sb_st = sbp.tile([128, 512], BF16, tag="sb_st")
                        for j in range(n // 128):
                            kk = eco // 128 + j
                            oh = sbp.tile([128, 128], BF16, tag="oh")
                            nc.vector.tensor_scalar(
                                oh[:], ramp128[:], arow_sb[:, kk:kk + 1], None,
                                mybir.AluOpType.is_equal)
                            nc.vector.tensor_scalar(
                                sb_s[:, 128 * j:128 * j + 128], ramp128[:],
                                arow_sb[:, kk:kk + 1], mask_sb[:, kk:kk + 1],
                                mybir.AluOpType.is_equal, mybir.AluOpType.mult)
                            nc.tensor.transpose(ps_tr[:, 128 * j:128 * j + 128],
                                                oh[:], w["i128b"][:])
                        nc.vector.tensor_copy(sb_st[:, 0:n], ps_tr[:, 0:n])
                        nc.sync.dma_start(ssd[:, 2 * eco:2 * eco + n], sb_st[:, 0:n])
                        nc.sync.dma_start(ssd[:, 2 * eco + n:2 * eco + 2 * n],
                                          sb_s[:, 0:n])

            # ---- layers ----
            for l in range(L):
                with tc.tile_pool(name=f"sbe{l}", bufs=3) as sbp, \
                     tc.tile_pool(name=f"pse{l}", bufs=2, space="PSUM") as psp, \
                     tc.tile_pool(name=f"psg{l}", bufs=2, space="PSUM") as psg:
                    for blk in blocks:
                        b = blk["b"]
                        hr = slice(64, 128) if blk["half"] else slice(0, 64)
                        ba = min(128, ASH - 128 * b)
                        asrc_full = asrc_fulls[l]
                        gt = sbp.tile([128, 1, blk["nblk_e"]], BF16, tag="gt", bufs=2)
                        if blk["n_lo"] and not no_gather:
                            io = sum(bb["n_lo"] for bb in blocks[:b]) // 16
                            nc.gpsimd.dma_gather(
                                gt[:, :, 0:blk["n_lo"]], asrc_full[0:LOS, :],
                                idxsb[:, io:io + blk["n_lo"] // 16],
                                blk["n_lo"], blk["n_lo"], ND, transpose=True,
                                queue_num=(2 * b) % 4)
                        if blk["n_hi"] and not no_gather:
                            io = sum(bb["n_hi"] for bb in blocks[:b]) // 16
                            nc.gpsimd.dma_gather(
                                gt[:, :, blk["n_lo"]:], asrc_full[LOS:N, :],
                                idxsbh[:, io:io + blk["n_hi"] // 16],
                                blk["n_hi"], blk["n_hi"], ND, transpose=True,
                                queue_num=(2 * b + 1) % 4)
                        hr2 = slice(64, 128) if b % 2 else slice(0, 64)
                        ps_aggT = psg.tile([128, 128], F32, tag="agg")
                        nchunk = len(blk["chunks"])
                        e0 = blk["chunks"][0][1]
                        ssb = sbp.tile([128, 2 * blk["nblk_e"]], BF16, tag="ssb", bufs=2)
                        nc.sync.dma_start(ssb[:, 0:2 * blk["nblk_e"]],
                                          ssd[:, 2 * e0:2 * e0 + 2 * blk["nblk_e"]])
                        for ci, (sco, eco, bco, n) in enumerate(blk["chunks"]):
                            sst = ssb[:, 2 * (eco - e0):2 * (eco - e0) + 2 * n]
                            ps_h = psp.tile([128, 512], F32, tag="ph")
                            nc.tensor.matmul(ps_h[:, 0:n], adst[0:ba, 128 * b:128 * b + 128],
                                             sst[0:ba, 0:n], start=True, stop=False)  # S^T chunk
                            nc.tensor.matmul(ps_h[:, 0:n], w["we_dup"][hr, l * HID:(l + 1) * HID],
                                             stateT[hr, sco:sco + n], start=False,
                                             stop=True)
                            pre = sbp.tile([128, 512], BF16, tag="pre")
                            if not no_gather:
                                nc.vector.scalar_tensor_tensor(
                                    pre[:, 0:n], ps_h[:, 0:n], w["eb1"][:, l:l + 1],
                                    gt[:, 0, bco:bco + n],
                                    mybir.AluOpType.add, mybir.AluOpType.add)
                            else:
                                nc.vector.tensor_scalar(pre[:, 0:n], ps_h[:, 0:n],
                                                        w["eb1"][:, l:l + 1], None,
                                                        mybir.AluOpType.add)
                            ht = sbp.tile([128, 512], BF16, tag="ht")
                            nc.scalar.activation(ht[:, 0:n], pre[:, 0:n], act)
                            ps_dd = psp.tile([128, 512], F32, tag="pd")
                            nc.tensor.matmul(ps_dd[hr, 0:n], w["ew2_all"][:, l * ED:(l + 1) * ED],
                                             ht[:, 0:n], start=True, stop=True)
                            nc.vector.scalar_tensor_tensor(
                                stateT[hr, sco:sco + n], ps_dd[hr, 0:n],
                                w["eb2p"][hr, l:l + 1], stateT[hr, sco:sco + n],
                                mybir.AluOpType.add, mybir.AluOpType.add)
                            ps_t = psp.tile([128, 256], BF16, tag="pt", bufs=1)
                            for j in range(n // 128):
                                nc.tensor.transpose(
                                    ps_t[:, 64 * j:64 * j + 64],
                                    stateT[hr, sco + 128 * j:sco + 128 * j + 128],
                                    w["i64d"][hr, :])
                            nn = sbp.tile([128, 256], BF16, tag="nn")
                            nc.vector.tensor_copy(nn[:, 0:64 * (n // 128)], ps_t[:, 0:64 * (n // 128)])
                            for j in range(n // 128):
                                nc.tensor.matmul(
                                    ps_aggT[hr2, :],
                                    nn[:, 64 * j:64 * j + 64],
                                    sst[:, n + 128 * j:n + 128 * j + 128],
                                    start=(ci == 0 and j == 0),
                                    stop=(ci == nchunk - 1 and j == n // 128 - 1))
                        nc.vector.tensor_copy(aggT[hr2, (b // 2) * 128:(b // 2) * 128 + 128],
                                              ps_aggT[hr2, :])
                # node MLP + next-layer tables
                with tc.tile_pool(name=f"sbn{l}", bufs=3) as sbp, \
                     tc.tile_pool(name=f"psn{l}", bufs=2, space="PSUM") as psp:
                    for t in range(NBLK):
                        na = min(128, ASH - 128 * t)
                        hr2 = slice(64, 128) if t % 2 else slice(0, 64)
                        ps_hn = psp.tile([128, 128], F32, tag="hn")
                        nc.tensor.matmul(ps_hn[:, 0:na],
                                         w["nw1a_all"][:, l * HID:(l + 1) * HID],
                                         nodeTb[:, 128 * t:128 * t + na],
                                         start=True, stop=False)
                        nc.tensor.matmul(ps_hn[:, 0:na],
                                         w["nw1b_dup"][hr2, l * HID:(l + 1) * HID],
                                         aggT[hr2, (t // 2) * 128:(t // 2) * 128 + na],
                                         start=False, stop=True)
                        hn = sbp.tile([128, 128], BF16, tag="hn_s")
                        nc.scalar.activation(hn[:, 0:na], ps_hn[:, 0:na], act,
                                             bias=w["nb1"][:, l:l + 1])
                        ps_nd = psp.tile([128, 128], F32, tag="ndl")
                        nc.tensor.matmul(ps_nd[:, 0:na],
                                         w["nw2_all"][:, l * ND:(l + 1) * ND],
                                         hn[:, 0:na], start=True, stop=True)
                        nc.vector.scalar_tensor_tensor(
                            nodeT[:, 128 * t:128 * t + na], ps_nd[:, 0:na],
                            w["nb2"][:, l:l + 1], nodeT[:, 128 * t:128 * t + na],
                            mybir.AluOpType.add, mybir.AluOpType.add)
                        nc.vector.tensor_copy(nodeTb[:, 128 * t:128 * t + na],
                                              nodeT[:, 128 * t:128 * t + na])
                    if l < L - 1:
                        node_tables(l + 1, sbp, psp)

            # ---- pooling ----
            with tc.tile_pool(name="sbt", bufs=3) as sbt, \
                 tc.tile_pool(name="pst", bufs=2, space="PSUM") as pst:
                nnat_all = pp.tile([128, NBLK * 128], BF16, tag="nnat_all")
                for t in range(NBLK):
                    na = min(128, ASH - 128 * t)
                    ps_tr = pst.tile([128, 128], F32, tag="ptr")
                    nc.tensor.transpose(ps_tr[0:na, :], nodeT[:, 128 * t:128 * t + na],
                                        w["i128f"][:])
                    nc.vector.tensor_copy(nnat_all[0:na, 128 * t:128 * t + 128][:, 0:128],
                                          ps_tr[0:na, :])
            with tc.tile_pool(name="sbp", bufs=3) as sbp, \
                 tc.tile_pool(name="psp", bufs=1, space="PSUM") as psp:
                pools = [psp.tile([128, 128], F32, tag=f"pool{g}", name=f"pool{g}") for g in range(NG)]
                rampP = sbp.tile([128, NG * GC], F32, tag="rampP", bufs=1)
                nc.gpsimd.iota(rampP[:], pattern=[[1, NG * GC]], base=0,
                               channel_multiplier=0,
                               allow_small_or_imprecise_dtypes=True)
                cbloc_sb = sbp.tile([128, NBLK], F32, tag="cbloc_sb", bufs=1)
                nc.sync.dma_start(cbloc_sb[:], cblocd[:])
                for t in range(NBLK):
                    na = min(128, ASH - 128 * t)
                    pmt = sbp.tile([128, NG * GC], BF16, tag="pmt")
                    nc.vector.tensor_scalar(pmt[:], rampP[:], cbloc_sb[:, t:t + 1],
                                            None, mybir.AluOpType.is_equal)
                    for g in range(NG):
                        gc = min(GC, NCRYS - g * GC)
                        nc.tensor.matmul(pools[g][0:gc, :], pmt[0:na, g * GC:g * GC + gc],
                                         nnat_all[0:na, 128 * t:128 * t + 128][:, 0:128],
                                         start=(t == 0), stop=(t == NBLK - 1))
                for g in range(NG):
                    gc = min(GC, NCRYS - g * GC)
                    pev = sbp.tile([128, 128], F32, tag="pev")
                    nc.vector.tensor_copy(pev[0:gc, :], pools[g][0:gc, :])
                    nc.sync.dma_start(pool_in[g * GC:g * GC + gc, :], pev[0:gc, :])
                nc.gpsimd.collective_compute(
                    "AllReduce", mybir.AluOpType.add,
                    replica_groups=[list(range(NCORES))],
                    ins=[pool_in[:].opt()], outs=[pool_out[:].opt()],
                )

            # ---- readout (replicated) ----
            with tc.tile_pool(name="sbr", bufs=2) as sbp, \
                 tc.tile_pool(name="psr", bufs=2, space="PSUM") as psp:
                for g in range(NG):
                    gc = min(GC, NCRYS - g * GC)
                    pg = sbp.tile([128, 128], F32, tag="pg")
                    nc.sync.dma_start(pg[0:gc, :], pool_out[g * GC:g * GC + gc, :])
                    mean = sbp.tile([128, 128], BF16, tag="mean")
                    nc.scalar.activation(mean[0:gc, :], pg[0:gc, :], AFT.Identity,
                                         scale=w["invccnt"][0:gc, g:g + 1])
                    ps_mt = psp.tile([128, 128], BF16, tag="pmt2")
                    nc.tensor.transpose(ps_mt[:, 0:gc], mean[0:gc, :], w["i128b"][0:gc, 0:gc])
                    meanT = sbp.tile([128, 128], BF16, tag="meanT")
                    nc.vector.tensor_copy(meanT[:, 0:gc], ps_mt[:, 0:gc])
                    ps_hr = psp.tile([128, 128], F32, tag="phr")
                    nc.tensor.matmul(ps_hr[:, 0:gc], w["readW"][:], meanT[:, 0:gc],
                                     start=True, stop=True)
                    hrT = sbp.tile([128, 128], BF16, tag="hrT")
                    nc.scalar.activation(hrT[:, 0:gc], ps_hr[:, 0:gc], act,
                                         bias=w["readb"][:])
                    ps_y = psp.tile([128, 128], F32, tag="py")
                    nc.tensor.matmul(ps_y[0:1, 0:gc], w["outW"][:], hrT[:, 0:gc],
                                     start=True, stop=True)
                    ysb = sbp.tile([1, 128], F32, tag="ysb")
                    nc.scalar.activation(ysb[0:1, 0:gc], ps_y[0:1, 0:gc], AFT.Copy,
                                         bias=meta["out_b"])
                    nc.sync.dma_start(y[0:1, g * GC:g * GC + gc], ysb[0:1, 0:gc])

    nc.compile()
    return nc


def run_cores(meta, in_maps, act=AFT.Silu, sim=False):
    nc = _build(meta, act=act)
    if sim:
        from concourse.bass_interp import MultiCoreSim
        s = MultiCoreSim(nc, NCORES, trace=False)
        for k in range(NCORES):
            for nm, arr in in_maps[k].items():
                s.cores[k].tensor(nm)[:] = arr
        s.simulate(check_with_hw=False)
        return [{"y": np.array(s.cores[k].tensor("y"))} for k in range(NCORES)], None
    from concourse import bass_utils
    res = bass_utils.run_bass_kernel_spmd(nc, in_maps, core_ids=list(range(NCORES)))
    return res.results, res


def kernel(**inputs):
    cfg = dict(FULL_CFG)
    n, m = np.asarray(inputs["nbr_fea_idx"]).shape
    cfg["N"], cfg["M"] = int(n), int(m)
    cfg["AFD"] = int(np.asarray(inputs["atom_fea"]).shape[1])
    cfg["EFD"] = int(np.asarray(inputs["nbr_fea"]).shape[2])
    cfg["NCRYS"] = int(inputs["num_crystals"])
    cfg["L"] = int(np.asarray(inputs["eW1"]).shape[0])
    meta, in_maps = _prep(inputs, cfg)
    results, _ = run_cores(meta, in_maps)
    return np.asarray(results[0]["y"], np.float32).reshape(cfg["NCRYS"], 1)

